# revision 5
# baseline (speedup 1.0000x reference)
"""Trainium2 Bass kernel for nn_Conv_hg_90022514524500 (hypergraph GNN).

Self-contained: hardcodes problem shapes, shards across 8 NeuronCores,
runs one SPMD Bass/Tile program via bass_utils.run_bass_kernel_spmd.
"""
import numpy as np

import concourse.bass as bass
import concourse.bacc as bacc
import concourse.tile as tile
import concourse.mybir as mybir
from concourse import bass_utils

F32 = mybir.dt.float32
I32 = mybir.dt.int32
AF = mybir.ActivationFunctionType
OP = mybir.AluOpType

N_EV, N_OBJ, D = 50000, 100000, 256
E1, E2 = 400000, 800000
NC = 8
EV_SH = N_EV // NC            # 6250
OBJ_SH = N_OBJ // NC          # 12500
NODE_SH = (N_EV + N_OBJ) // NC  # 18750
EDGE_SH = N_EV // NC          # 6250
EV_W = (EV_SH + 127) // 128       # 49
OBJ_W = (OBJ_SH + 127) // 128     # 98
NODE_W = (NODE_SH + 127) // 128   # 147
EDGE_W = (EDGE_SH + 127) // 128   # 49
OBJ_PAD = OBJ_W * 128             # 12544
NODE_PAD = NODE_W * 128           # 18816
PAD_L = 200.0                     # one-hot "no segment" sentinel

N_NODE = N_EV + N_OBJ
XW1_ROWS = NC * (N_EV // NC + (N_OBJ // NC + 127) // 128 * 128)  # 150352
XW2_ROWS = NC * NODE_PAD          # 150528: padded node rows


XW1_CH = EV_SH + OBJ_PAD   # 18794 rows per rank in XW1_full


def _xw1_row(node):
    """Row of node's features in XW1_full (per-rank [evXW | objXW] chunks)."""
    node = np.asarray(node)
    obj = node - N_EV
    return np.where(node < N_EV,
                    XW1_CH * (node // EV_SH) + node % EV_SH,
                    XW1_CH * (obj // OBJ_SH) + EV_SH + obj % OBJ_SH).astype(np.int64)


def _xw2_row(node):
    node = np.asarray(node)
    return (NODE_PAD * (node // NODE_SH) + node % NODE_SH).astype(np.int64)


def _build_streams(seg, n_seg_per_core, n_win, payload_cols):
    """Group incidences by core (seg // n_seg_per_core), sort by seg, pack into
    SPMD-uniform (128 x C) slot arrays chunked per 128-segment window.

    payload_cols: dict name -> per-incidence int64/float array (original order).
    Returns nch[n_win], C, dict name -> [NC,128,C] arrays, l [NC,128,C] f32,
    orig [NC,128,C] int64 (original incidence idx, -1 pads).
    """
    n = seg.shape[0]
    core = seg // n_seg_per_core
    local = seg - core * n_seg_per_core
    win = local // 128
    # counts[c, w]
    counts = np.zeros((NC, n_win), np.int64)
    np.add.at(counts, (core, win), 1)
    nch = np.maximum(1, (counts.max(0) + 127) // 128)  # per window, shared by cores
    base = np.concatenate([[0], np.cumsum(nch)])       # chunk base per window
    C = int(base[-1])

    out = {k: np.zeros((NC, 128, C), v.dtype) for k, v in payload_cols.items()}
    l_arr = np.full((NC, 128, C), PAD_L, np.float32)
    orig = np.full((NC, 128, C), -1, np.int64)

    order = np.lexsort((local, core))   # by core, then by local seg (stable)
    core_s = core[order]
    local_s = local[order]
    win_s = win[order]
    # position within (core, window): running index over sorted groups
    key = core_s * n_win + win_s
    start_of_group = np.concatenate([[True], key[1:] != key[:-1]])
    group_id = np.cumsum(start_of_group) - 1
    group_first = np.where(start_of_group)[0]
    pwin = np.arange(n) - group_first[group_id]       # 0..count-1 within group
    col = base[win_s] + pwin // 128
    p = pwin % 128
    l_arr[core_s, p, col] = (local_s - 128 * win_s).astype(np.float32)
    orig[core_s, p, col] = order
    for k, v in payload_cols.items():
        out[k][core_s, p, col] = v[order]
    return nch, C, out, l_arr, orig


def _prep(inputs):
    """All host-side preprocessing. Returns (meta, per-core input maps pieces)."""
    oe_ev = np.asarray(inputs['oe_ev']).astype(np.int64)
    oe_obj = np.asarray(inputs['oe_obj']).astype(np.int64)
    hg_node = np.asarray(inputs['hg_node']).astype(np.int64)
    hg_edge = np.asarray(inputs['hg_edge']).astype(np.int64)

    meta = {}
    # oe stream: segments = objects
    nchO, C_O, oeP, oeL, _ = _build_streams(
        oe_obj, OBJ_SH, OBJ_W, {'idx': oe_ev.astype(np.int32)})
    # pass A stream: segments = edges; payload = node (converted per layer)
    nchA, C_A, hgAP, hgAL, _ = _build_streams(
        hg_edge, EDGE_SH, EDGE_W,
        {'idx1': _xw1_row(hg_node).astype(np.int32),
         'idx2': _xw2_row(hg_node).astype(np.int32)})
    # pass B stream: segments = nodes; payload = edge (ef row)
    nchB, C_B, hgBP, hgBL, origB = _build_streams(
        hg_node, NODE_SH, NODE_W, {'idx': hg_edge.astype(np.int32)})

    meta['nchO'], meta['C_O'] = nchO, C_O
    meta['nchA'], meta['C_A'] = nchA, C_A
    meta['nchB'], meta['C_B'] = nchB, C_B
    meta['maxB'] = int(nchB.max())
    meta['maxA'] = int(nchA.max())
    meta['maxO'] = int(nchO.max())

    # edge mean reciprocal counts, padded per core [EDGE_W*128]
    cnt = np.bincount(hg_edge, minlength=N_EV).astype(np.float32)
    rcnt = 1.0 / np.maximum(cnt, 1.0)
    rcnt_c = np.ones((NC, EDGE_W * 128, 1), np.float32)
    for c in range(NC):
        rcnt_c[c, :EDGE_SH, 0] = rcnt[c * EDGE_SH:(c + 1) * EDGE_SH]

    # t1 gather rows for L1 pass B windows: [NC, 128, NODE_W]
    t1idx = np.zeros((NC, 128, NODE_W), np.int32)
    for c in range(NC):
        ln = np.arange(NODE_PAD)
        node = c * NODE_SH + np.minimum(ln, NODE_SH - 1)
        rows = _xw1_row(node).astype(np.int32)
        t1idx[c] = rows.reshape(NODE_W, 128).T

    # input shards
    evX = np.asarray(inputs['event_X'], np.float32)
    objX = np.asarray(inputs['object_X'], np.float32)
    evX_c = np.zeros((NC, EV_W * 128, D), np.float32)
    objX_c = np.zeros((NC, OBJ_PAD, D), np.float32)
    for c in range(NC):
        evX_c[c, :EV_SH] = evX[c * EV_SH:(c + 1) * EV_SH]
        objX_c[c, :OBJ_SH] = objX[c * OBJ_SH:(c + 1) * OBJ_SH]

    # replicated constant rows
    names = ['be', 'ge', 'ben', 'bo', 'go', 'bon', 'bu', 'g1', 'b1',
             'bl', 'g2', 'b2']
    vecs = [np.asarray(inputs[k], np.float32) for k in names]
    vecs.append(np.asarray(inputs['ah1'], np.float32)[D:])
    vecs.append(np.asarray(inputs['ah2'], np.float32)[D:])
    brep = np.tile(np.concatenate(vecs)[None, :], (128, 1)).astype(np.float32)
    meta['brep_names'] = names + ['a2_1', 'a2_2']

    def halves(w):
        w = np.asarray(w, np.float32)
        return w[:128, :].copy(), w[128:, :].copy()

    Wh1a = np.concatenate([np.asarray(inputs['Wh1'], np.float32),
                           (np.asarray(inputs['Wh1'], np.float32)
                            @ np.asarray(inputs['ah1'], np.float32)[:D])[:, None]], 1)
    Wh2a = np.concatenate([np.asarray(inputs['Wh2'], np.float32),
                           (np.asarray(inputs['Wh2'], np.float32)
                            @ np.asarray(inputs['ah2'], np.float32)[:D])[:, None]], 1)
    weights = {}
    for nm, w in [('wo', inputs['Wo']), ('we', inputs['We']),
                  ('wu', inputs['Wu']), ('wl', inputs['Wl']),
                  ('wh1', Wh1a), ('wh2', Wh2a)]:
        weights[nm + '0'], weights[nm + '1'] = halves(w)

    ident = np.eye(128, dtype=np.float32)
    iotaR = np.tile(np.arange(128, dtype=np.float32)[None, :], (128, 1))

    in_maps = []
    for c in range(NC):
        m = {
            'evX': evX_c[c], 'objX': objX_c[c],
            'oe_idx': oeP['idx'][c], 'oe_l': oeL[c],
            'hgA1_idx': hgAP['idx1'][c], 'hgA2_idx': hgAP['idx2'][c],
            'hgA_l': hgAL[c],
            'hgB_idx': hgBP['idx'][c], 'hgB_l': hgBL[c],
            'hgT1_idx': t1idx[c],
            'rcnt': rcnt_c[c],
            'brep': brep, 'ident': ident, 'iotaR': iotaR,
        }
        m.update(weights)
        in_maps.append(m)
    return meta, in_maps, origB


def _build(meta):
    nchO, nchA, nchB = meta['nchO'], meta['nchA'], meta['nchB']
    C_O, C_A, C_B = meta['C_O'], meta['C_A'], meta['C_B']

    nc = bacc.Bacc("TRN2", target_bir_lowering=False, debug=False,
                   enable_asserts=False, num_devices=NC)

    def din(name, shape, dt=F32):
        return nc.dram_tensor(name, shape, dt, kind="ExternalInput").ap()

    evX = din('evX', [EV_W * 128, D])
    objX = din('objX', [OBJ_PAD, D])
    oe_idx = din('oe_idx', [128, C_O], I32)
    oe_l = din('oe_l', [128, C_O])
    hgA1_idx = din('hgA1_idx', [128, C_A], I32)
    hgA2_idx = din('hgA2_idx', [128, C_A], I32)
    hgA_l = din('hgA_l', [128, C_A])
    hgB_idx = din('hgB_idx', [128, C_B], I32)
    hgB_l = din('hgB_l', [128, C_B])
    hgT1_idx = din('hgT1_idx', [128, NODE_W], I32)
    rcnt = din('rcnt', [EDGE_W * 128, 1])
    brep_i = din('brep', [128, 14 * D])
    ident_i = din('ident', [128, 128])
    iota_i = din('iotaR', [128, 128])
    wts = {nm: din(nm, [128, D]) for nm in
           ['wo0', 'wo1', 'we0', 'we1', 'wu0', 'wu1', 'wl0', 'wl1']}
    wts.update({nm: din(nm, [128, D + 1]) for nm in
                ['wh10', 'wh11', 'wh20', 'wh21']})

    h_out = nc.dram_tensor('h_out', [NODE_PAD, D], F32, kind="ExternalOutput").ap()
    attn_out = nc.dram_tensor('attn_out', [128, C_B], F32, kind="ExternalOutput").ap()

    BN = ['be', 'ge', 'ben', 'bo', 'go', 'bon', 'bu', 'g1', 'b1',
          'bl', 'g2', 'b2', 'a2_1', 'a2_2']

    from contextlib import ExitStack
    with tile.TileContext(nc) as tc, ExitStack() as es:
        consts = es.enter_context(tc.tile_pool(name="consts", bufs=1))
        dram = es.enter_context(tc.tile_pool(name="dram", bufs=1, space="DRAM"))
        io = es.enter_context(tc.tile_pool(name="io", bufs=3))
        trp = es.enter_context(tc.tile_pool(name="trp", bufs=4))
        gp = es.enter_context(tc.tile_pool(name="gp", bufs=6))
        sm = es.enter_context(tc.tile_pool(name="sm", bufs=4))
        ps_tr = es.enter_context(tc.tile_pool(name="ps_tr", bufs=2, space="PSUM"))
        ps_mm = es.enter_context(tc.tile_pool(name="ps_mm", bufs=2, space="PSUM"))
        ps_w = es.enter_context(tc.tile_pool(name="ps_w", bufs=2, space="PSUM"))

        # resident constants
        ident = consts.tile([128, 128], F32)
        nc.sync.dma_start(out=ident[:], in_=ident_i[:])
        iotaR = consts.tile([128, 128], F32)
        nc.sync.dma_start(out=iotaR[:], in_=iota_i[:])
        brep = consts.tile([128, 14 * D], F32)
        nc.sync.dma_start(out=brep[:], in_=brep_i[:])
        bv = {nm: brep[:, i * D:(i + 1) * D] for i, nm in enumerate(BN)}
        w = {}
        for nm, ap in wts.items():
            t = consts.tile([128, ap.shape[1]], F32, name=f"w_{nm}")
            nc.sync.dma_start(out=t[:], in_=ap[:])
            w[nm] = t
        eps = consts.tile([128, 1], F32)
        nc.vector.memset(eps[:], 1e-5)

        # DRAM intermediates
        evAG_in = dram.tile([EV_SH, D], F32)
        ev_full = dram.tile([N_EV, D], F32, addr_space="Shared")
        xw1_in = dram.tile([EV_SH + OBJ_PAD, D + 1], F32)
        xw1_full = dram.tile([XW1_ROWS, D + 1], F32, addr_space="Shared")
        obj_c = dram.tile([OBJ_PAD, D], F32)
        ef_in = dram.tile([EDGE_SH, D + 1], F32)
        ef_full = dram.tile([N_EV, D + 1], F32, addr_space="Shared")
        h1_c = dram.tile([NODE_PAD, D], F32)
        xw2_in = dram.tile([NODE_PAD, D + 1], F32)
        xw2_full = dram.tile([XW2_ROWS, D + 1], F32, addr_space="Shared")
        ef2_in = dram.tile([EDGE_SH, D + 1], F32)
        ef2_full = dram.tile([N_EV, D + 1], F32, addr_space="Shared")

        def lrelu_inplace(t, tmp_pool=sm):
            tmp = tmp_pool.tile([128, t.shape[1]], F32, name="lr_tmp", tag="lr_tmp")
            nc.scalar.mul(out=tmp[:], in_=t[:], mul=0.2)
            nc.vector.tensor_tensor(out=t[:], in0=t[:], in1=tmp[:], op=OP.max)

        def layernorm(y, gR, bR):
            stats = sm.tile([128, 6], F32, name="ln_stats", tag="ln_stats")
            nc.vector.bn_stats(out=stats[:], in_=y[:])
            mv = sm.tile([128, 2], F32, name="ln_mv", tag="ln_mv")
            nc.vector.bn_aggr(out=mv[:], in_=stats[:])
            std = sm.tile([128, 1], F32, name="ln_std", tag="ln_std")
            nc.scalar.activation(out=std[:], in_=mv[:, 1:2], func=AF.Sqrt, bias=eps[:])
            rstd = sm.tile([128, 1], F32, name="ln_rstd", tag="ln_rstd")
            nc.vector.reciprocal(out=rstd[:], in_=std[:])
            nc.vector.tensor_scalar(out=y[:], in0=y[:], scalar1=mv[:, 0:1],
                                    scalar2=rstd[:], op0=OP.subtract, op1=OP.mult)
            nc.vector.tensor_tensor(out=y[:], in0=y[:], in1=gR, op=OP.mult)
            nc.vector.tensor_tensor(out=y[:], in0=y[:], in1=bR, op=OP.add)

        def transpose2(x_tile):
            """[128,256] -> two [128,128] transposed tiles (d on partitions)."""
            pt = ps_tr.tile([128, 256], F32, name="trps", tag="tr")
            xts = []
            for k in range(2):
                nc.tensor.transpose(out=pt[:, 128 * k:128 * (k + 1)],
                                    in_=x_tile[:, 128 * k:128 * (k + 1)],
                                    identity=ident[:])
                st = trp.tile([128, 128], F32, name=f"trsb{k}", tag=f"trsb{k}")
                nc.vector.tensor_copy(out=st[:], in_=pt[:, 128 * k:128 * (k + 1)])
                xts.append(st)
            return xts

        def matmul_w(xts, w0, w1, ncols):
            pm = ps_w.tile([128, ncols], F32, name="mmw", tag="mmw")
            nc.tensor.matmul(out=pm[:], lhsT=xts[0][:], rhs=w0[:, :ncols],
                             start=True, stop=False)
            nc.tensor.matmul(out=pm[:], lhsT=xts[1][:], rhs=w1[:, :ncols],
                             start=False, stop=True)
            return pm

        def proj_block(x_tile, w0, w1, bR, gR, btR):
            """LN(lrelu(x@W + b)) * g + bt -> sbuf tile [128, D]"""
            xts = transpose2(x_tile)
            pm = matmul_w(xts, w0, w1, D)
            y = io.tile([128, D], F32, name="proj_y", tag="proj_y")
            nc.vector.tensor_tensor(out=y[:], in0=pm[:], in1=bR, op=OP.add)
            lrelu_inplace(y)
            layernorm(y, gR, btR)
            return y

        # ---------------- P1: event projection + evXW ----------------
        for wi in range(EV_W):
            rows = min(128, EV_SH - 128 * wi)
            xt = io.tile([128, D], F32, name="evx", tag="evx")
            nc.sync.dma_start(out=xt[:], in_=evX[128 * wi:128 * (wi + 1), :])
            ev_t = proj_block(xt, w['we0'], w['we1'], bv['be'], bv['ge'], bv['ben'])
            nc.sync.dma_start(out=evAG_in[128 * wi:128 * wi + rows, :],
                              in_=ev_t[:rows, :])
            xts = transpose2(ev_t)
            pm = matmul_w(xts, w['wh10'], w['wh11'], D + 1)
            xw_t = io.tile([128, D + 1], F32, name="xw_t", tag="xw_t")
            nc.vector.tensor_copy(out=xw_t[:], in_=pm[:])
            nc.sync.dma_start(out=xw1_in[128 * wi:128 * wi + rows, :],
                              in_=xw_t[:rows, :])

        # ---------------- P2: AllGather ev + evXW ----------------
        nc.gpsimd.collective_compute(
            "AllGather", OP.bypass, replica_groups=[list(range(NC))],
            ins=[evAG_in[:]], outs=[ev_full[:]])


        # ---------------- P3: object projection ----------------
        for wi in range(OBJ_W):
            xt = io.tile([128, D], F32, name="objx", tag="objx")
            nc.sync.dma_start(out=xt[:], in_=objX[128 * wi:128 * (wi + 1), :])
            ob_t = proj_block(xt, w['wo0'], w['wo1'], bv['bo'], bv['go'], bv['bon'])
            nc.sync.dma_start(out=obj_c[128 * wi:128 * (wi + 1), :], in_=ob_t[:])

        # ---------------- P4: oe segment sum + obj1/obj2 + objXW ----------------
        pos = 0
        for wi in range(OBJ_W):
            pmsg = ps_mm.tile([128, D], F32, name="pmsg", tag="acc")
            nch = int(nchO[wi])
            oix = sm.tile([128, meta['maxO']], I32, name="oe_ix", tag="oe_ix")
            nc.sync.dma_start(out=oix[:, :nch], in_=oe_idx[:, pos:pos + nch])
            for j in range(nch):
                g = gp.tile([128, D], F32, name="oe_g", tag="oe_g")
                nc.gpsimd.indirect_dma_start(
                    out=g[:], out_offset=None, in_=ev_full[:],
                    in_offset=bass.IndirectOffsetOnAxis(
                        ap=oix[:, j:j + 1], axis=0))
                lt = sm.tile([128, 1], F32, name="oe_lt", tag="oe_lt")
                nc.sync.dma_start(out=lt[:], in_=oe_l[:, pos + j:pos + j + 1])
                P = trp.tile([128, 128], F32, name="oe_P", tag="oe_P")
                nc.vector.tensor_scalar(out=P[:], in0=iotaR[:], scalar1=lt[:],
                                        scalar2=None, op0=OP.is_equal)
                nc.tensor.matmul(out=pmsg[:], lhsT=P[:], rhs=g[:],
                                 start=(j == 0), stop=(j == nch - 1))
            pos += nch
            # obj1 = LN(lrelu(msg@Wu + bu) + obj) * g1 + b1
            msg = io.tile([128, D], F32, name="msg", tag="msg")
            nc.vector.tensor_copy(out=msg[:], in_=pmsg[:])
            xts = transpose2(msg)
            pm1 = matmul_w(xts, w['wu0'], w['wu1'], D)
            y1 = io.tile([128, D], F32, name="y1", tag="y1")
            nc.vector.tensor_tensor(out=y1[:], in0=pm1[:], in1=bv['bu'], op=OP.add)
            lrelu_inplace(y1)
            ob = io.tile([128, D], F32, name="ob_in", tag="ob_in")
            nc.sync.dma_start(out=ob[:], in_=obj_c[128 * wi:128 * (wi + 1), :])
            nc.vector.tensor_tensor(out=y1[:], in0=y1[:], in1=ob[:], op=OP.add)
            layernorm(y1, bv['g1'], bv['b1'])
            # obj2 = LN(lrelu(obj1@Wl + bl) + obj1) * g2 + b2
            xts = transpose2(y1)
            pm2 = matmul_w(xts, w['wl0'], w['wl1'], D)
            y2 = io.tile([128, D], F32, name="y2", tag="y2")
            nc.vector.tensor_tensor(out=y2[:], in0=pm2[:], in1=bv['bl'], op=OP.add)
            lrelu_inplace(y2)
            nc.vector.tensor_tensor(out=y2[:], in0=y2[:], in1=y1[:], op=OP.add)
            layernorm(y2, bv['g2'], bv['b2'])
            # objXW
            xts = transpose2(y2)
            pm3 = matmul_w(xts, w['wh10'], w['wh11'], D + 1)
            xw_t = io.tile([128, D + 1], F32, name="oxw_t", tag="oxw_t")
            nc.vector.tensor_copy(out=xw_t[:], in_=pm3[:])
            nc.sync.dma_start(out=xw1_in[EV_SH + 128 * wi:EV_SH + 128 * (wi + 1), :],
                              in_=xw_t[:])

        # ---------------- P5: AllGather XW1 ----------------
        nc.gpsimd.collective_compute(
            "AllGather", OP.bypass, replica_groups=[list(range(NC))],
            ins=[xw1_in[:]], outs=[xw1_full[:]])

        def pass_a(idx_ap, src_full, ef_dst, a2R):
            pos = 0
            for wi in range(EDGE_W):
                rows = min(128, EDGE_SH - 128 * wi)
                pA = ps_mm.tile([128, D + 1], F32, name="pA", tag="acc")
                nch = int(nchA[wi])
                aix = sm.tile([128, meta['maxA']], I32, name="A_ix", tag="A_ix")
                nc.sync.dma_start(out=aix[:, :nch], in_=idx_ap[:, pos:pos + nch])
                for j in range(nch):
                    g = gp.tile([128, D + 1], F32, name="A_g", tag="A_g")
                    nc.gpsimd.indirect_dma_start(
                        out=g[:], out_offset=None, in_=src_full[:],
                        in_offset=bass.IndirectOffsetOnAxis(
                            ap=aix[:, j:j + 1], axis=0))
                    lt = sm.tile([128, 1], F32, name="A_lt", tag="A_lt")
                    nc.sync.dma_start(out=lt[:], in_=hgA_l[:, pos + j:pos + j + 1])
                    P = trp.tile([128, 128], F32, name="A_P", tag="A_P")
                    nc.vector.tensor_scalar(out=P[:], in0=iotaR[:], scalar1=lt[:],
                                            scalar2=None, op0=OP.is_equal)
                    nc.tensor.matmul(out=pA[:], lhsT=P[:], rhs=g[:],
                                     start=(j == 0), stop=(j == nch - 1))
                pos += nch
                rc = sm.tile([128, 1], F32, name="A_rc", tag="A_rc")
                nc.sync.dma_start(out=rc[:], in_=rcnt[128 * wi:128 * (wi + 1), :])
                ef_t = io.tile([128, D + 1], F32, name="ef_t", tag="ef_t")
                nc.vector.tensor_scalar_mul(out=ef_t[:, :D], in0=pA[:, :D],
                                            scalar1=rc[:])
                scr = trp.tile([128, D], F32, name="A_scr", tag="A_scr")
                nc.vector.tensor_tensor(out=scr[:], in0=ef_t[:, :D], in1=a2R,
                                        op=OP.mult)
                nc.vector.tensor_reduce(out=ef_t[:, D:D + 1], in_=scr[:],
                                        axis=mybir.AxisListType.X, op=OP.add)
                nc.sync.dma_start(out=ef_dst[128 * wi:128 * wi + rows, :],
                                  in_=ef_t[:rows, :])

        def pass_b(ef_src, t1_mode, xw_fuse, attn_tile):
            pos = 0
            maxB = meta['maxB']
            for wi in range(NODE_W):
                # t1 window column
                t1c = sm.tile([128, 1], F32, name="B_t1c", tag="B_t1c")
                if t1_mode == 'gather':
                    tix = sm.tile([128, 1], I32, name="B_tix", tag="B_tix")
                    nc.sync.dma_start(out=tix[:], in_=hgT1_idx[:, wi:wi + 1])
                    nc.gpsimd.indirect_dma_start(
                        out=t1c[:], out_offset=None, in_=xw1_full[:],
                        in_offset=bass.IndirectOffsetOnAxis(
                            ap=tix[:], axis=0),
                        element_offset=D)
                else:
                    nc.sync.dma_start(
                        out=t1c[:], in_=xw2_in[128 * wi:128 * (wi + 1), D:D + 1])
                ptr = ps_tr.tile([128, 128], F32, name="B_t1ps", tag="tr")
                nc.tensor.transpose(out=ptr[:], in_=t1c[:].to_broadcast([128, 128]),
                                    identity=ident[:])
                t1R = trp.tile([128, 128], F32, name="B_t1R", tag="B_t1R")
                nc.vector.tensor_copy(out=t1R[:], in_=ptr[:])

                nch = int(nchB[wi])
                bix = sm.tile([128, maxB], I32, name="B_ix", tag="B_ix")
                nc.sync.dma_start(out=bix[:, :nch], in_=hgB_idx[:, pos:pos + nch])
                Ps, gths = [], []
                scores = sm.tile([128, maxB], F32, name="B_sc", tag="B_sc")
                for j in range(nch):
                    g = gp.tile([128, D + 1], F32, name=f"B_g{j}", tag=f"B_g{j}")
                    nc.gpsimd.indirect_dma_start(
                        out=g[:], out_offset=None, in_=ef_src[:],
                        in_offset=bass.IndirectOffsetOnAxis(
                            ap=bix[:, j:j + 1], axis=0))
                    gths.append(g)
                    lt = sm.tile([128, 1], F32, name="B_lt", tag="B_lt")
                    nc.sync.dma_start(out=lt[:], in_=hgB_l[:, pos + j:pos + j + 1])
                    P = trp.tile([128, 128], F32, name=f"B_P{j}", tag=f"B_P{j}")
                    nc.vector.tensor_scalar(out=P[:], in0=iotaR[:], scalar1=lt[:],
                                            scalar2=None, op0=OP.is_equal)
                    Ps.append(P)
                    scr = trp.tile([128, 128], F32, name="B_scr", tag="B_scr")
                    nc.vector.tensor_tensor(out=scr[:], in0=P[:], in1=t1R[:],
                                            op=OP.mult)
                    t1i = sm.tile([128, 1], F32, name="B_t1i", tag="B_t1i")
                    nc.vector.tensor_reduce(out=t1i[:], in_=scr[:],
                                            axis=mybir.AxisListType.X, op=OP.add)
                    nc.vector.tensor_tensor(out=scores[:, j:j + 1], in0=t1i[:],
                                            in1=g[:, D:D + 1], op=OP.add)
                # lrelu + exp over the window's scores
                lrelu_inplace(scores[:, :nch])
                eT = sm.tile([128, maxB], F32, name="B_e", tag="B_e")
                nc.scalar.activation(out=eT[:, :nch], in_=scores[:, :nch], func=AF.Exp)
                pB = ps_mm.tile([128, D + 1], F32, name="pB", tag="acc")
                for j in range(nch):
                    rhs = gp.tile([128, D + 1], F32, name="B_rhs", tag="B_rhs")
                    nc.vector.tensor_scalar_mul(out=rhs[:, :D], in0=gths[j][:, :D],
                                                scalar1=eT[:, j:j + 1])
                    nc.vector.tensor_copy(out=rhs[:, D:D + 1], in_=eT[:, j:j + 1])
                    nc.tensor.matmul(out=pB[:], lhsT=Ps[j][:], rhs=rhs[:],
                                     start=(j == 0), stop=(j == nch - 1))
                # flush
                zc = sm.tile([128, 1], F32, name="B_zc", tag="B_zc")
                nc.vector.tensor_scalar(out=zc[:], in0=pB[:, D:D + 1], scalar1=1e-9,
                                        scalar2=None, op0=OP.max)
                rz = sm.tile([128, 1], F32, name="B_rz", tag="B_rz")
                nc.vector.reciprocal(out=rz[:], in_=zc[:])
                h_t = io.tile([128, D], F32, name="B_h", tag="B_h")
                nc.vector.tensor_scalar_mul(out=h_t[:], in0=pB[:, :D], scalar1=rz[:])
                lrelu_inplace(h_t)
                if xw_fuse:
                    nc.sync.dma_start(out=h1_c[128 * wi:128 * (wi + 1), :], in_=h_t[:])
                    xts = transpose2(h_t)
                    pmx = matmul_w(xts, w['wh20'], w['wh21'], D + 1)
                    xw_t = io.tile([128, D + 1], F32, name="B_xw", tag="B_xw")
                    nc.vector.tensor_copy(out=xw_t[:], in_=pmx[:])
                    nc.sync.dma_start(out=xw2_in[128 * wi:128 * (wi + 1), :],
                                      in_=xw_t[:])
                else:
                    nc.sync.dma_start(out=h_out[128 * wi:128 * (wi + 1), :],
                                      in_=h_t[:])
                    # attention per incidence: e * rz[node]
                    prz = ps_tr.tile([128, 128], F32, name="B_rzps", tag="tr")
                    nc.tensor.transpose(out=prz[:],
                                        in_=rz[:].to_broadcast([128, 128]),
                                        identity=ident[:])
                    rzR = trp.tile([128, 128], F32, name="B_rzR", tag="B_rzR")
                    nc.vector.tensor_copy(out=rzR[:], in_=prz[:])
                    for j in range(nch):
                        scr = trp.tile([128, 128], F32, name="B_scr2", tag="B_scr2")
                        nc.vector.tensor_tensor(out=scr[:], in0=Ps[j][:],
                                                in1=rzR[:], op=OP.mult)
                        rzi = sm.tile([128, 1], F32, name="B_rzi", tag="B_rzi")
                        nc.vector.tensor_reduce(out=rzi[:], in_=scr[:],
                                                axis=mybir.AxisListType.X, op=OP.add)
                        nc.vector.tensor_tensor(out=attn_tile[:, pos + j:pos + j + 1],
                                                in0=rzi[:], in1=eT[:, j:j + 1],
                                                op=OP.mult)
                pos += nch

        a2_1 = bv['a2_1']
        a2_2 = bv['a2_2']
        attn_t = consts.tile([128, C_B], F32)

        # ---------------- Layer 1 ----------------
        pass_a(hgA1_idx, xw1_full, ef_in, a2_1)
        nc.gpsimd.collective_compute(
            "AllGather", OP.bypass, replica_groups=[list(range(NC))],
            ins=[ef_in[:]], outs=[ef_full[:]])
        pass_b(ef_full, 'gather', True, attn_t)
        nc.gpsimd.collective_compute(
            "AllGather", OP.bypass, replica_groups=[list(range(NC))],
            ins=[xw2_in[:]], outs=[xw2_full[:]])

        # ---------------- Layer 2 ----------------
        pass_a(hgA2_idx, xw2_full, ef2_in, a2_2)
        nc.gpsimd.collective_compute(
            "AllGather", OP.bypass, replica_groups=[list(range(NC))],
            ins=[ef2_in[:]], outs=[ef2_full[:]])
        pass_b(ef2_full, 'local', False, attn_t)

        nc.sync.dma_start(out=attn_out[:], in_=attn_t[:])

    nc.compile()
    return nc


def kernel(**inputs):
    meta, in_maps, origB = _prep(inputs)
    nc = _build(meta)
    res = bass_utils.run_bass_kernel_spmd(nc, in_maps, core_ids=list(range(NC)))
    h = np.zeros((N_NODE, D), np.float32)
    attn = np.zeros((E2,), np.float32)
    for c in range(NC):
        h[c * NODE_SH:(c + 1) * NODE_SH] = res.results[c]['h_out'][:NODE_SH]
        a = res.results[c]['attn_out']
        m = origB[c]
        valid = m >= 0
        attn[m[valid]] = a[valid]
    return h, attn


# revision 6
# speedup vs baseline: 1.1340x; 1.1340x over previous
"""Trainium2 Bass kernel for nn_Conv_hg_90022514524500 (hypergraph GNN).

Self-contained: hardcodes problem shapes, shards across 8 NeuronCores,
runs one SPMD Bass/Tile program via bass_utils.run_bass_kernel_spmd.
"""
import numpy as np

import concourse.bass as bass
import concourse.bacc as bacc
import concourse.tile as tile
import concourse.mybir as mybir
from concourse import bass_utils

F32 = mybir.dt.float32
I32 = mybir.dt.int32
AF = mybir.ActivationFunctionType
OP = mybir.AluOpType

N_EV, N_OBJ, D = 50000, 100000, 256
E1, E2 = 400000, 800000
NC = 8
EV_SH = N_EV // NC            # 6250
OBJ_SH = N_OBJ // NC          # 12500
NODE_SH = (N_EV + N_OBJ) // NC  # 18750
EDGE_SH = N_EV // NC          # 6250
EV_W = (EV_SH + 127) // 128       # 49
OBJ_W = (OBJ_SH + 127) // 128     # 98
NODE_W = (NODE_SH + 127) // 128   # 147
EDGE_W = (EDGE_SH + 127) // 128   # 49
OBJ_PAD = OBJ_W * 128             # 12544
NODE_PAD = NODE_W * 128           # 18816
PAD_L = 200.0                     # one-hot "no segment" sentinel

N_NODE = N_EV + N_OBJ
XW1_ROWS = NC * (N_EV // NC + (N_OBJ // NC + 127) // 128 * 128)  # 150352
XW2_ROWS = NC * NODE_PAD          # 150528: padded node rows


XW1_CH = EV_SH + OBJ_PAD   # 18794 rows per rank in XW1_full


def _xw1_row(node):
    """Row of node's features in XW1_full (per-rank [evXW | objXW] chunks)."""
    node = np.asarray(node)
    obj = node - N_EV
    return np.where(node < N_EV,
                    XW1_CH * (node // EV_SH) + node % EV_SH,
                    XW1_CH * (obj // OBJ_SH) + EV_SH + obj % OBJ_SH).astype(np.int64)


def _xw2_row(node):
    node = np.asarray(node)
    return (NODE_PAD * (node // NODE_SH) + node % NODE_SH).astype(np.int64)


def _build_streams(seg, n_seg_per_core, n_win, payload_cols):
    """Group incidences by core (seg // n_seg_per_core), sort by seg, pack into
    SPMD-uniform (128 x C) slot arrays chunked per 128-segment window.

    payload_cols: dict name -> per-incidence int64/float array (original order).
    Returns nch[n_win], C, dict name -> [NC,128,C] arrays, l [NC,128,C] f32,
    orig [NC,128,C] int64 (original incidence idx, -1 pads).
    """
    n = seg.shape[0]
    core = seg // n_seg_per_core
    local = seg - core * n_seg_per_core
    win = local // 128
    # counts[c, w]
    counts = np.zeros((NC, n_win), np.int64)
    np.add.at(counts, (core, win), 1)
    nch = np.maximum(1, (counts.max(0) + 127) // 128)  # per window, shared by cores
    base = np.concatenate([[0], np.cumsum(nch)])       # chunk base per window
    C = int(base[-1])

    out = {k: np.zeros((NC, 128, C), v.dtype) for k, v in payload_cols.items()}
    l_arr = np.full((NC, 128, C), PAD_L, np.float32)
    orig = np.full((NC, 128, C), -1, np.int64)

    order = np.lexsort((local, core))   # by core, then by local seg (stable)
    core_s = core[order]
    local_s = local[order]
    win_s = win[order]
    # position within (core, window): running index over sorted groups
    key = core_s * n_win + win_s
    start_of_group = np.concatenate([[True], key[1:] != key[:-1]])
    group_id = np.cumsum(start_of_group) - 1
    group_first = np.where(start_of_group)[0]
    pwin = np.arange(n) - group_first[group_id]       # 0..count-1 within group
    col = base[win_s] + pwin // 128
    p = pwin % 128
    l_arr[core_s, p, col] = (local_s - 128 * win_s).astype(np.float32)
    orig[core_s, p, col] = order
    for k, v in payload_cols.items():
        out[k][core_s, p, col] = v[order]
    return nch, C, out, l_arr, orig


def _prep(inputs):
    """All host-side preprocessing. Returns (meta, per-core input maps pieces)."""
    oe_ev = np.asarray(inputs['oe_ev']).astype(np.int64)
    oe_obj = np.asarray(inputs['oe_obj']).astype(np.int64)
    hg_node = np.asarray(inputs['hg_node']).astype(np.int64)
    hg_edge = np.asarray(inputs['hg_edge']).astype(np.int64)

    meta = {}
    # oe stream: segments = objects
    nchO, C_O, oeP, oeL, _ = _build_streams(
        oe_obj, OBJ_SH, OBJ_W, {'idx': oe_ev.astype(np.int32)})
    # pass A stream: segments = edges; payload = node (converted per layer)
    nchA, C_A, hgAP, hgAL, _ = _build_streams(
        hg_edge, EDGE_SH, EDGE_W,
        {'idx1': _xw1_row(hg_node).astype(np.int32),
         'idx2': _xw2_row(hg_node).astype(np.int32)})
    # pass B stream: segments = nodes; payload = edge (ef row)
    nchB, C_B, hgBP, hgBL, origB = _build_streams(
        hg_node, NODE_SH, NODE_W, {'idx': hg_edge.astype(np.int32)})

    meta['nchO'], meta['C_O'] = nchO, C_O
    meta['nchA'], meta['C_A'] = nchA, C_A
    meta['nchB'], meta['C_B'] = nchB, C_B
    meta['maxB'] = int(nchB.max())
    meta['maxA'] = int(nchA.max())
    meta['maxO'] = int(nchO.max())

    # edge mean reciprocal counts, padded per core [EDGE_W*128]
    cnt = np.bincount(hg_edge, minlength=N_EV).astype(np.float32)
    rcnt = 1.0 / np.maximum(cnt, 1.0)
    rcnt_c = np.ones((NC, EDGE_W * 128, 1), np.float32)
    for c in range(NC):
        rcnt_c[c, :EDGE_SH, 0] = rcnt[c * EDGE_SH:(c + 1) * EDGE_SH]

    # t1 gather rows for L1 pass B windows: [NC, 128, NODE_W]
    t1idx = np.zeros((NC, 128, NODE_W), np.int32)
    for c in range(NC):
        ln = np.arange(NODE_PAD)
        node = c * NODE_SH + np.minimum(ln, NODE_SH - 1)
        rows = _xw1_row(node).astype(np.int32)
        t1idx[c] = rows.reshape(NODE_W, 128).T

    # input shards
    evX = np.asarray(inputs['event_X'], np.float32)
    objX = np.asarray(inputs['object_X'], np.float32)
    evX_c = np.zeros((NC, EV_W * 128, D), np.float32)
    objX_c = np.zeros((NC, OBJ_PAD, D), np.float32)
    for c in range(NC):
        evX_c[c, :EV_SH] = evX[c * EV_SH:(c + 1) * EV_SH]
        objX_c[c, :OBJ_SH] = objX[c * OBJ_SH:(c + 1) * OBJ_SH]

    # replicated constant rows
    names = ['be', 'ge', 'ben', 'bo', 'go', 'bon', 'bu', 'g1', 'b1',
             'bl', 'g2', 'b2']
    vecs = [np.asarray(inputs[k], np.float32) for k in names]
    vecs.append(np.asarray(inputs['ah1'], np.float32)[D:])
    vecs.append(np.asarray(inputs['ah2'], np.float32)[D:])
    brep = np.tile(np.concatenate(vecs)[None, :], (128, 1)).astype(np.float32)
    meta['brep_names'] = names + ['a2_1', 'a2_2']

    def halves(w):
        w = np.asarray(w, np.float32)
        return w[:128, :].copy(), w[128:, :].copy()

    Wh1a = np.concatenate([np.asarray(inputs['Wh1'], np.float32),
                           (np.asarray(inputs['Wh1'], np.float32)
                            @ np.asarray(inputs['ah1'], np.float32)[:D])[:, None]], 1)
    Wh2a = np.concatenate([np.asarray(inputs['Wh2'], np.float32),
                           (np.asarray(inputs['Wh2'], np.float32)
                            @ np.asarray(inputs['ah2'], np.float32)[:D])[:, None]], 1)
    weights = {}
    for nm, w in [('wo', inputs['Wo']), ('we', inputs['We']),
                  ('wu', inputs['Wu']), ('wl', inputs['Wl']),
                  ('wh1', Wh1a), ('wh2', Wh2a)]:
        weights[nm + '0'], weights[nm + '1'] = halves(w)

    ident = np.eye(128, dtype=np.float32)
    iotaR = np.tile(np.arange(128, dtype=np.float32)[None, :], (128, 1))

    in_maps = []
    for c in range(NC):
        m = {
            'evX': evX_c[c], 'objX': objX_c[c],
            'oe_idx': oeP['idx'][c], 'oe_l': oeL[c],
            'hgA1_idx': hgAP['idx1'][c], 'hgA2_idx': hgAP['idx2'][c],
            'hgA_l': hgAL[c],
            'hgB_idx': hgBP['idx'][c], 'hgB_l': hgBL[c],
            'hgT1_idx': t1idx[c],
            'rcnt': rcnt_c[c],
            'brep': brep, 'ident': ident, 'iotaR': iotaR,
        }
        m.update(weights)
        in_maps.append(m)
    return meta, in_maps, origB


def _build(meta):
    nchO, nchA, nchB = meta['nchO'], meta['nchA'], meta['nchB']
    C_O, C_A, C_B = meta['C_O'], meta['C_A'], meta['C_B']

    nc = bacc.Bacc("TRN2", target_bir_lowering=False, debug=False,
                   enable_asserts=False, num_devices=NC)

    def din(name, shape, dt=F32):
        return nc.dram_tensor(name, shape, dt, kind="ExternalInput").ap()

    evX = din('evX', [EV_W * 128, D])
    objX = din('objX', [OBJ_PAD, D])
    oe_idx = din('oe_idx', [128, C_O], I32)
    oe_l = din('oe_l', [128, C_O])
    hgA1_idx = din('hgA1_idx', [128, C_A], I32)
    hgA2_idx = din('hgA2_idx', [128, C_A], I32)
    hgA_l = din('hgA_l', [128, C_A])
    hgB_idx = din('hgB_idx', [128, C_B], I32)
    hgB_l = din('hgB_l', [128, C_B])
    hgT1_idx = din('hgT1_idx', [128, NODE_W], I32)
    rcnt = din('rcnt', [EDGE_W * 128, 1])
    brep_i = din('brep', [128, 14 * D])
    ident_i = din('ident', [128, 128])
    iota_i = din('iotaR', [128, 128])
    wts = {nm: din(nm, [128, D]) for nm in
           ['wo0', 'wo1', 'we0', 'we1', 'wu0', 'wu1', 'wl0', 'wl1']}
    wts.update({nm: din(nm, [128, D + 1]) for nm in
                ['wh10', 'wh11', 'wh20', 'wh21']})

    h_out = nc.dram_tensor('h_out', [NODE_PAD, D], F32, kind="ExternalOutput").ap()
    attn_out = nc.dram_tensor('attn_out', [128, C_B], F32, kind="ExternalOutput").ap()

    BN = ['be', 'ge', 'ben', 'bo', 'go', 'bon', 'bu', 'g1', 'b1',
          'bl', 'g2', 'b2', 'a2_1', 'a2_2']

    from contextlib import ExitStack
    with tile.TileContext(nc) as tc, ExitStack() as es:
        consts = es.enter_context(tc.tile_pool(name="consts", bufs=1))
        dram = es.enter_context(tc.tile_pool(name="dram", bufs=1, space="DRAM"))
        io = es.enter_context(tc.tile_pool(name="io", bufs=3))
        trp = es.enter_context(tc.tile_pool(name="trp", bufs=4))
        gp = es.enter_context(tc.tile_pool(name="gp", bufs=10))
        sm = es.enter_context(tc.tile_pool(name="sm", bufs=6))
        ps_tr = es.enter_context(tc.tile_pool(name="ps_tr", bufs=2, space="PSUM"))
        ps_mm = es.enter_context(tc.tile_pool(name="ps_mm", bufs=2, space="PSUM"))
        ps_w = es.enter_context(tc.tile_pool(name="ps_w", bufs=2, space="PSUM"))

        # resident constants
        ident = consts.tile([128, 128], F32)
        nc.sync.dma_start(out=ident[:], in_=ident_i[:])
        iotaR = consts.tile([128, 128], F32)
        nc.sync.dma_start(out=iotaR[:], in_=iota_i[:])
        brep = consts.tile([128, 14 * D], F32)
        nc.sync.dma_start(out=brep[:], in_=brep_i[:])
        bv = {nm: brep[:, i * D:(i + 1) * D] for i, nm in enumerate(BN)}
        w = {}
        for nm, ap in wts.items():
            t = consts.tile([128, ap.shape[1]], F32, name=f"w_{nm}")
            nc.sync.dma_start(out=t[:], in_=ap[:])
            w[nm] = t
        eps = consts.tile([128, 1], F32)
        nc.vector.memset(eps[:], 1e-5)

        # DRAM intermediates
        evAG_in = dram.tile([EV_SH, D], F32)
        ev_full = dram.tile([N_EV, D], F32, addr_space="Shared")
        xw1_in = dram.tile([EV_SH + OBJ_PAD, D + 1], F32)
        xw1_full = dram.tile([XW1_ROWS, D + 1], F32, addr_space="Shared")
        obj_c = dram.tile([OBJ_PAD, D], F32)
        ef_in = dram.tile([EDGE_SH, D + 1], F32)
        ef_full = dram.tile([N_EV, D + 1], F32, addr_space="Shared")
        h1_c = dram.tile([NODE_PAD, D], F32)
        xw2_in = dram.tile([NODE_PAD, D + 1], F32)
        xw2_full = dram.tile([XW2_ROWS, D + 1], F32, addr_space="Shared")
        ef2_in = dram.tile([EDGE_SH, D + 1], F32)
        ef2_full = dram.tile([N_EV, D + 1], F32, addr_space="Shared")

        def lrelu_inplace(t, tmp_pool=sm):
            tmp = tmp_pool.tile([128, t.shape[1]], F32, name="lr_tmp", tag="lr_tmp")
            nc.scalar.mul(out=tmp[:], in_=t[:], mul=0.2)
            nc.vector.tensor_tensor(out=t[:], in0=t[:], in1=tmp[:], op=OP.max)

        def layernorm(y, gR, bR):
            stats = sm.tile([128, 6], F32, name="ln_stats", tag="ln_stats")
            nc.vector.bn_stats(out=stats[:], in_=y[:])
            mv = sm.tile([128, 2], F32, name="ln_mv", tag="ln_mv")
            nc.vector.bn_aggr(out=mv[:], in_=stats[:])
            std = sm.tile([128, 1], F32, name="ln_std", tag="ln_std")
            nc.scalar.activation(out=std[:], in_=mv[:, 1:2], func=AF.Sqrt, bias=eps[:])
            rstd = sm.tile([128, 1], F32, name="ln_rstd", tag="ln_rstd")
            nc.vector.reciprocal(out=rstd[:], in_=std[:])
            nc.vector.tensor_scalar(out=y[:], in0=y[:], scalar1=mv[:, 0:1],
                                    scalar2=rstd[:], op0=OP.subtract, op1=OP.mult)
            nc.vector.tensor_tensor(out=y[:], in0=y[:], in1=gR, op=OP.mult)
            nc.vector.tensor_tensor(out=y[:], in0=y[:], in1=bR, op=OP.add)

        def transpose2(x_tile):
            """[128,256] -> two [128,128] transposed tiles (d on partitions)."""
            pt = ps_tr.tile([128, 256], F32, name="trps", tag="tr")
            xts = []
            for k in range(2):
                nc.tensor.transpose(out=pt[:, 128 * k:128 * (k + 1)],
                                    in_=x_tile[:, 128 * k:128 * (k + 1)],
                                    identity=ident[:])
                st = trp.tile([128, 128], F32, name=f"trsb{k}", tag=f"trsb{k}")
                nc.vector.tensor_copy(out=st[:], in_=pt[:, 128 * k:128 * (k + 1)])
                xts.append(st)
            return xts

        def matmul_w(xts, w0, w1, ncols):
            pm = ps_w.tile([128, ncols], F32, name="mmw", tag="mmw")
            nc.tensor.matmul(out=pm[:], lhsT=xts[0][:], rhs=w0[:, :ncols],
                             start=True, stop=False)
            nc.tensor.matmul(out=pm[:], lhsT=xts[1][:], rhs=w1[:, :ncols],
                             start=False, stop=True)
            return pm

        def proj_block(x_tile, w0, w1, bR, gR, btR):
            """LN(lrelu(x@W + b)) * g + bt -> sbuf tile [128, D]"""
            xts = transpose2(x_tile)
            pm = matmul_w(xts, w0, w1, D)
            y = io.tile([128, D], F32, name="proj_y", tag="proj_y")
            nc.vector.tensor_tensor(out=y[:], in0=pm[:], in1=bR, op=OP.add)
            lrelu_inplace(y)
            layernorm(y, gR, btR)
            return y

        # ---------------- P1: event projection + evXW ----------------
        for wi in range(EV_W):
            rows = min(128, EV_SH - 128 * wi)
            xt = io.tile([128, D], F32, name="evx", tag="evx")
            nc.sync.dma_start(out=xt[:], in_=evX[128 * wi:128 * (wi + 1), :])
            ev_t = proj_block(xt, w['we0'], w['we1'], bv['be'], bv['ge'], bv['ben'])
            nc.sync.dma_start(out=evAG_in[128 * wi:128 * wi + rows, :],
                              in_=ev_t[:rows, :])
            xts = transpose2(ev_t)
            pm = matmul_w(xts, w['wh10'], w['wh11'], D + 1)
            xw_t = io.tile([128, D + 1], F32, name="xw_t", tag="xw_t")
            nc.vector.tensor_copy(out=xw_t[:], in_=pm[:])
            nc.sync.dma_start(out=xw1_in[128 * wi:128 * wi + rows, :],
                              in_=xw_t[:rows, :])

        # ---------------- P2: AllGather ev + evXW ----------------
        nc.gpsimd.collective_compute(
            "AllGather", OP.bypass, replica_groups=[list(range(NC))],
            ins=[evAG_in[:]], outs=[ev_full[:]])


        # ---------------- P3: object projection ----------------
        for wi in range(OBJ_W):
            xt = io.tile([128, D], F32, name="objx", tag="objx")
            nc.sync.dma_start(out=xt[:], in_=objX[128 * wi:128 * (wi + 1), :])
            ob_t = proj_block(xt, w['wo0'], w['wo1'], bv['bo'], bv['go'], bv['bon'])
            nc.sync.dma_start(out=obj_c[128 * wi:128 * (wi + 1), :], in_=ob_t[:])

        # ---------------- P4: oe segment sum + obj1/obj2 + objXW ----------------
        pos = 0
        for wi in range(OBJ_W):
            pmsg = ps_mm.tile([128, D], F32, name="pmsg", tag="acc")
            nch = int(nchO[wi])
            oix = sm.tile([128, meta['maxO']], I32, name="oe_ix", tag="oe_ix")
            nc.sync.dma_start(out=oix[:, :nch], in_=oe_idx[:, pos:pos + nch])
            olt = sm.tile([128, meta['maxO']], F32, name="oe_lw", tag="oe_lw")
            nc.sync.dma_start(out=olt[:, :nch], in_=oe_l[:, pos:pos + nch])
            for j in range(nch):
                g = gp.tile([128, D], F32, name="oe_g", tag="oe_g")
                nc.gpsimd.indirect_dma_start(
                    out=g[:], out_offset=None, in_=ev_full[:],
                    in_offset=bass.IndirectOffsetOnAxis(
                        ap=oix[:, j:j + 1], axis=0))
                P = trp.tile([128, 128], F32, name="oe_P", tag="oe_P")
                nc.vector.tensor_scalar(out=P[:], in0=iotaR[:], scalar1=olt[:, j:j + 1],
                                        scalar2=None, op0=OP.is_equal)
                nc.tensor.matmul(out=pmsg[:], lhsT=P[:], rhs=g[:],
                                 start=(j == 0), stop=(j == nch - 1))
            pos += nch
            # obj1 = LN(lrelu(msg@Wu + bu) + obj) * g1 + b1
            msg = io.tile([128, D], F32, name="msg", tag="msg")
            nc.vector.tensor_copy(out=msg[:], in_=pmsg[:])
            xts = transpose2(msg)
            pm1 = matmul_w(xts, w['wu0'], w['wu1'], D)
            y1 = io.tile([128, D], F32, name="y1", tag="y1")
            nc.vector.tensor_tensor(out=y1[:], in0=pm1[:], in1=bv['bu'], op=OP.add)
            lrelu_inplace(y1)
            ob = io.tile([128, D], F32, name="ob_in", tag="ob_in")
            nc.sync.dma_start(out=ob[:], in_=obj_c[128 * wi:128 * (wi + 1), :])
            nc.vector.tensor_tensor(out=y1[:], in0=y1[:], in1=ob[:], op=OP.add)
            layernorm(y1, bv['g1'], bv['b1'])
            # obj2 = LN(lrelu(obj1@Wl + bl) + obj1) * g2 + b2
            xts = transpose2(y1)
            pm2 = matmul_w(xts, w['wl0'], w['wl1'], D)
            y2 = io.tile([128, D], F32, name="y2", tag="y2")
            nc.vector.tensor_tensor(out=y2[:], in0=pm2[:], in1=bv['bl'], op=OP.add)
            lrelu_inplace(y2)
            nc.vector.tensor_tensor(out=y2[:], in0=y2[:], in1=y1[:], op=OP.add)
            layernorm(y2, bv['g2'], bv['b2'])
            # objXW
            xts = transpose2(y2)
            pm3 = matmul_w(xts, w['wh10'], w['wh11'], D + 1)
            xw_t = io.tile([128, D + 1], F32, name="oxw_t", tag="oxw_t")
            nc.vector.tensor_copy(out=xw_t[:], in_=pm3[:])
            nc.sync.dma_start(out=xw1_in[EV_SH + 128 * wi:EV_SH + 128 * (wi + 1), :],
                              in_=xw_t[:])

        # ---------------- P5: AllGather XW1 ----------------
        nc.gpsimd.collective_compute(
            "AllGather", OP.bypass, replica_groups=[list(range(NC))],
            ins=[xw1_in[:]], outs=[xw1_full[:]])

        def pass_a(idx_ap, src_full, ef_dst, a2R):
            pos = 0
            for wi in range(EDGE_W):
                rows = min(128, EDGE_SH - 128 * wi)
                pA = ps_mm.tile([128, D + 1], F32, name="pA", tag="acc")
                nch = int(nchA[wi])
                aix = sm.tile([128, meta['maxA']], I32, name="A_ix", tag="A_ix")
                nc.sync.dma_start(out=aix[:, :nch], in_=idx_ap[:, pos:pos + nch])
                alw = sm.tile([128, meta['maxA']], F32, name="A_lw", tag="A_lw")
                nc.sync.dma_start(out=alw[:, :nch], in_=hgA_l[:, pos:pos + nch])
                for j in range(nch):
                    g = gp.tile([128, D + 1], F32, name="A_g", tag="A_g")
                    nc.gpsimd.indirect_dma_start(
                        out=g[:], out_offset=None, in_=src_full[:],
                        in_offset=bass.IndirectOffsetOnAxis(
                            ap=aix[:, j:j + 1], axis=0))
                    P = trp.tile([128, 128], F32, name="A_P", tag="A_P")
                    nc.vector.tensor_scalar(out=P[:], in0=iotaR[:], scalar1=alw[:, j:j + 1],
                                            scalar2=None, op0=OP.is_equal)
                    nc.tensor.matmul(out=pA[:], lhsT=P[:], rhs=g[:],
                                     start=(j == 0), stop=(j == nch - 1))
                pos += nch
                rc = sm.tile([128, 1], F32, name="A_rc", tag="A_rc")
                nc.sync.dma_start(out=rc[:], in_=rcnt[128 * wi:128 * (wi + 1), :])
                ef_t = io.tile([128, D + 1], F32, name="ef_t", tag="ef_t")
                nc.vector.tensor_scalar_mul(out=ef_t[:, :D], in0=pA[:, :D],
                                            scalar1=rc[:])
                scr = trp.tile([128, D], F32, name="A_scr", tag="A_scr")
                nc.vector.tensor_tensor(out=scr[:], in0=ef_t[:, :D], in1=a2R,
                                        op=OP.mult)
                nc.vector.tensor_reduce(out=ef_t[:, D:D + 1], in_=scr[:],
                                        axis=mybir.AxisListType.X, op=OP.add)
                nc.sync.dma_start(out=ef_dst[128 * wi:128 * wi + rows, :],
                                  in_=ef_t[:rows, :])

        def pass_b(ef_src, t1_mode, xw_fuse, attn_tile):
            pos = 0
            maxB = meta['maxB']
            for wi in range(NODE_W):
                # t1 window column
                t1c = sm.tile([128, 1], F32, name="B_t1c", tag="B_t1c")
                if t1_mode == 'gather':
                    tix = sm.tile([128, 1], I32, name="B_tix", tag="B_tix")
                    nc.sync.dma_start(out=tix[:], in_=hgT1_idx[:, wi:wi + 1])
                    nc.gpsimd.indirect_dma_start(
                        out=t1c[:], out_offset=None, in_=xw1_full[:],
                        in_offset=bass.IndirectOffsetOnAxis(
                            ap=tix[:], axis=0),
                        element_offset=D)
                else:
                    nc.sync.dma_start(
                        out=t1c[:], in_=xw2_in[128 * wi:128 * (wi + 1), D:D + 1])
                ptr = ps_tr.tile([128, 128], F32, name="B_t1ps", tag="tr")
                nc.tensor.transpose(out=ptr[:], in_=t1c[:].to_broadcast([128, 128]),
                                    identity=ident[:])
                t1R = trp.tile([128, 128], F32, name="B_t1R", tag="B_t1R")
                nc.vector.tensor_copy(out=t1R[:], in_=ptr[:])

                nch = int(nchB[wi])
                bix = sm.tile([128, maxB], I32, name="B_ix", tag="B_ix")
                nc.sync.dma_start(out=bix[:, :nch], in_=hgB_idx[:, pos:pos + nch])
                blw = sm.tile([128, maxB], F32, name="B_lw", tag="B_lw")
                nc.sync.dma_start(out=blw[:, :nch], in_=hgB_l[:, pos:pos + nch])
                Ps, gths = [], []
                scores = sm.tile([128, maxB], F32, name="B_sc", tag="B_sc")
                for j in range(nch):
                    g = gp.tile([128, D + 1], F32, name=f"B_g{j}", tag=f"B_g{j}")
                    nc.gpsimd.indirect_dma_start(
                        out=g[:], out_offset=None, in_=ef_src[:],
                        in_offset=bass.IndirectOffsetOnAxis(
                            ap=bix[:, j:j + 1], axis=0))
                    gths.append(g)
                    P = trp.tile([128, 128], F32, name=f"B_P{j}", tag=f"B_P{j}")
                    nc.vector.tensor_scalar(out=P[:], in0=iotaR[:], scalar1=blw[:, j:j + 1],
                                            scalar2=None, op0=OP.is_equal)
                    Ps.append(P)
                    scr = trp.tile([128, 128], F32, name="B_scr", tag="B_scr")
                    nc.vector.tensor_tensor(out=scr[:], in0=P[:], in1=t1R[:],
                                            op=OP.mult)
                    t1i = sm.tile([128, 1], F32, name="B_t1i", tag="B_t1i")
                    nc.vector.tensor_reduce(out=t1i[:], in_=scr[:],
                                            axis=mybir.AxisListType.X, op=OP.add)
                    nc.vector.tensor_tensor(out=scores[:, j:j + 1], in0=t1i[:],
                                            in1=g[:, D:D + 1], op=OP.add)
                # lrelu + exp over the window's scores
                lrelu_inplace(scores[:, :nch])
                eT = sm.tile([128, maxB], F32, name="B_e", tag="B_e")
                nc.scalar.activation(out=eT[:, :nch], in_=scores[:, :nch], func=AF.Exp)
                pB = ps_mm.tile([128, D + 1], F32, name="pB", tag="acc")
                for j in range(nch):
                    rhs = gp.tile([128, D + 1], F32, name="B_rhs", tag="B_rhs")
                    nc.vector.tensor_scalar_mul(out=rhs[:, :D], in0=gths[j][:, :D],
                                                scalar1=eT[:, j:j + 1])
                    nc.vector.tensor_copy(out=rhs[:, D:D + 1], in_=eT[:, j:j + 1])
                    nc.tensor.matmul(out=pB[:], lhsT=Ps[j][:], rhs=rhs[:],
                                     start=(j == 0), stop=(j == nch - 1))
                # flush
                zc = sm.tile([128, 1], F32, name="B_zc", tag="B_zc")
                nc.vector.tensor_scalar(out=zc[:], in0=pB[:, D:D + 1], scalar1=1e-9,
                                        scalar2=None, op0=OP.max)
                rz = sm.tile([128, 1], F32, name="B_rz", tag="B_rz")
                nc.vector.reciprocal(out=rz[:], in_=zc[:])
                h_t = io.tile([128, D], F32, name="B_h", tag="B_h")
                nc.vector.tensor_scalar_mul(out=h_t[:], in0=pB[:, :D], scalar1=rz[:])
                lrelu_inplace(h_t)
                if xw_fuse:
                    xts = transpose2(h_t)
                    pmx = matmul_w(xts, w['wh20'], w['wh21'], D + 1)
                    xw_t = io.tile([128, D + 1], F32, name="B_xw", tag="B_xw")
                    nc.vector.tensor_copy(out=xw_t[:], in_=pmx[:])
                    nc.sync.dma_start(out=xw2_in[128 * wi:128 * (wi + 1), :],
                                      in_=xw_t[:])
                else:
                    nc.sync.dma_start(out=h_out[128 * wi:128 * (wi + 1), :],
                                      in_=h_t[:])
                    # attention per incidence: e * rz[node]
                    prz = ps_tr.tile([128, 128], F32, name="B_rzps", tag="tr")
                    nc.tensor.transpose(out=prz[:],
                                        in_=rz[:].to_broadcast([128, 128]),
                                        identity=ident[:])
                    rzR = trp.tile([128, 128], F32, name="B_rzR", tag="B_rzR")
                    nc.vector.tensor_copy(out=rzR[:], in_=prz[:])
                    for j in range(nch):
                        scr = trp.tile([128, 128], F32, name="B_scr2", tag="B_scr2")
                        nc.vector.tensor_tensor(out=scr[:], in0=Ps[j][:],
                                                in1=rzR[:], op=OP.mult)
                        rzi = sm.tile([128, 1], F32, name="B_rzi", tag="B_rzi")
                        nc.vector.tensor_reduce(out=rzi[:], in_=scr[:],
                                                axis=mybir.AxisListType.X, op=OP.add)
                        nc.vector.tensor_tensor(out=attn_tile[:, pos + j:pos + j + 1],
                                                in0=rzi[:], in1=eT[:, j:j + 1],
                                                op=OP.mult)
                pos += nch

        a2_1 = bv['a2_1']
        a2_2 = bv['a2_2']
        attn_t = consts.tile([128, C_B], F32)

        # ---------------- Layer 1 ----------------
        pass_a(hgA1_idx, xw1_full, ef_in, a2_1)
        nc.gpsimd.collective_compute(
            "AllGather", OP.bypass, replica_groups=[list(range(NC))],
            ins=[ef_in[:]], outs=[ef_full[:]])
        pass_b(ef_full, 'gather', True, attn_t)
        nc.gpsimd.collective_compute(
            "AllGather", OP.bypass, replica_groups=[list(range(NC))],
            ins=[xw2_in[:]], outs=[xw2_full[:]])

        # ---------------- Layer 2 ----------------
        pass_a(hgA2_idx, xw2_full, ef2_in, a2_2)
        nc.gpsimd.collective_compute(
            "AllGather", OP.bypass, replica_groups=[list(range(NC))],
            ins=[ef2_in[:]], outs=[ef2_full[:]])
        pass_b(ef2_full, 'local', False, attn_t)

        nc.sync.dma_start(out=attn_out[:], in_=attn_t[:])

    nc.compile()
    return nc


def kernel(**inputs):
    meta, in_maps, origB = _prep(inputs)
    nc = _build(meta)
    res = bass_utils.run_bass_kernel_spmd(nc, in_maps, core_ids=list(range(NC)))
    h = np.zeros((N_NODE, D), np.float32)
    attn = np.zeros((E2,), np.float32)
    for c in range(NC):
        h[c * NODE_SH:(c + 1) * NODE_SH] = res.results[c]['h_out'][:NODE_SH]
        a = res.results[c]['attn_out']
        m = origB[c]
        valid = m >= 0
        attn[m[valid]] = a[valid]
    return h, attn


# revision 8
# speedup vs baseline: 1.3315x; 1.1742x over previous
"""Trainium2 Bass kernel for nn_Conv_hg_90022514524500 (hypergraph GNN).

Self-contained: hardcodes problem shapes, shards across 8 NeuronCores,
runs one SPMD Bass/Tile program via bass_utils.run_bass_kernel_spmd.
"""
import numpy as np

import concourse.bass as bass
import concourse.bacc as bacc
import concourse.tile as tile
import concourse.mybir as mybir
from concourse import bass_utils

F32 = mybir.dt.float32
I32 = mybir.dt.int32
AF = mybir.ActivationFunctionType
OP = mybir.AluOpType

N_EV, N_OBJ, D = 50000, 100000, 256
E1, E2 = 400000, 800000
NC = 8
EV_SH = N_EV // NC            # 6250
OBJ_SH = N_OBJ // NC          # 12500
NODE_SH = (N_EV + N_OBJ) // NC  # 18750
EDGE_SH = N_EV // NC          # 6250
EV_W = (EV_SH + 127) // 128       # 49
OBJ_W = (OBJ_SH + 127) // 128     # 98
NODE_W = (NODE_SH + 127) // 128   # 147
EDGE_W = (EDGE_SH + 127) // 128   # 49
OBJ_PAD = OBJ_W * 128             # 12544
NODE_PAD = NODE_W * 128           # 18816
PAD_L = 200.0                     # one-hot "no segment" sentinel

N_NODE = N_EV + N_OBJ
XW1_ROWS = NC * (N_EV // NC + (N_OBJ // NC + 127) // 128 * 128)  # 150352
XW2_ROWS = NC * NODE_PAD          # 150528: padded node rows


XW1_CH = EV_SH + OBJ_PAD   # 18794 rows per rank in XW1_full


def _xw1_row(node):
    """Row of node's features in XW1_full (per-rank [evXW | objXW] chunks)."""
    node = np.asarray(node)
    obj = node - N_EV
    return np.where(node < N_EV,
                    XW1_CH * (node // EV_SH) + node % EV_SH,
                    XW1_CH * (obj // OBJ_SH) + EV_SH + obj % OBJ_SH).astype(np.int64)


def _xw2_row(node):
    node = np.asarray(node)
    return (NODE_PAD * (node // NODE_SH) + node % NODE_SH).astype(np.int64)


def _build_streams(seg, n_seg_per_core, n_win, payload_cols):
    """Group incidences by core (seg // n_seg_per_core), sort by seg, pack into
    SPMD-uniform (128 x C) slot arrays chunked per 128-segment window.

    payload_cols: dict name -> per-incidence int64/float array (original order).
    Returns nch[n_win], C, dict name -> [NC,128,C] arrays, l [NC,128,C] f32,
    orig [NC,128,C] int64 (original incidence idx, -1 pads).
    """
    n = seg.shape[0]
    core = seg // n_seg_per_core
    local = seg - core * n_seg_per_core
    win = local // 128
    # counts[c, w]
    counts = np.zeros((NC, n_win), np.int64)
    np.add.at(counts, (core, win), 1)
    nch = np.maximum(1, (counts.max(0) + 127) // 128)  # per window, shared by cores
    base = np.concatenate([[0], np.cumsum(nch)])       # chunk base per window
    C = int(base[-1])

    out = {k: np.zeros((NC, 128, C), v.dtype) for k, v in payload_cols.items()}
    l_arr = np.full((NC, 128, C), PAD_L, np.float32)
    orig = np.full((NC, 128, C), -1, np.int64)

    order = np.lexsort((local, core))   # by core, then by local seg (stable)
    core_s = core[order]
    local_s = local[order]
    win_s = win[order]
    # position within (core, window): running index over sorted groups
    key = core_s * n_win + win_s
    start_of_group = np.concatenate([[True], key[1:] != key[:-1]])
    group_id = np.cumsum(start_of_group) - 1
    group_first = np.where(start_of_group)[0]
    pwin = np.arange(n) - group_first[group_id]       # 0..count-1 within group
    col = base[win_s] + pwin // 128
    p = pwin % 128
    l_arr[core_s, p, col] = (local_s - 128 * win_s).astype(np.float32)
    orig[core_s, p, col] = order
    for k, v in payload_cols.items():
        out[k][core_s, p, col] = v[order]
    return nch, C, out, l_arr, orig


def _prep(inputs):
    """All host-side preprocessing. Returns (meta, per-core input maps pieces)."""
    oe_ev = np.asarray(inputs['oe_ev']).astype(np.int64)
    oe_obj = np.asarray(inputs['oe_obj']).astype(np.int64)
    hg_node = np.asarray(inputs['hg_node']).astype(np.int64)
    hg_edge = np.asarray(inputs['hg_edge']).astype(np.int64)

    meta = {}
    # oe stream: segments = objects
    nchO, C_O, oeP, oeL, _ = _build_streams(
        oe_obj, OBJ_SH, OBJ_W, {'idx': oe_ev.astype(np.int32)})
    # pass A stream: segments = edges; payload = node (converted per layer)
    nchA, C_A, hgAP, hgAL, _ = _build_streams(
        hg_edge, EDGE_SH, EDGE_W,
        {'idx1': _xw1_row(hg_node).astype(np.int32),
         'idx2': _xw2_row(hg_node).astype(np.int32)})
    # pass B stream: segments = nodes; payload = edge (ef row)
    nchB, C_B, hgBP, hgBL, origB = _build_streams(
        hg_node, NODE_SH, NODE_W, {'idx': hg_edge.astype(np.int32)})

    meta['nchO'], meta['C_O'] = nchO, C_O
    meta['nchA'], meta['C_A'] = nchA, C_A
    meta['nchB'], meta['C_B'] = nchB, C_B
    meta['maxB'] = int(nchB.max())
    meta['maxA'] = int(nchA.max())
    meta['maxO'] = int(nchO.max())

    # edge mean reciprocal counts, padded per core [EDGE_W*128]
    cnt = np.bincount(hg_edge, minlength=N_EV).astype(np.float32)
    rcnt = 1.0 / np.maximum(cnt, 1.0)
    rcnt_c = np.ones((NC, EDGE_W * 128, 1), np.float32)
    for c in range(NC):
        rcnt_c[c, :EDGE_SH, 0] = rcnt[c * EDGE_SH:(c + 1) * EDGE_SH]

    # t1 gather rows for L1 pass B windows: [NC, 128, NODE_W]
    t1idx = np.zeros((NC, 128, NODE_W), np.int32)
    for c in range(NC):
        ln = np.arange(NODE_PAD)
        node = c * NODE_SH + np.minimum(ln, NODE_SH - 1)
        rows = _xw1_row(node).astype(np.int32)
        t1idx[c] = rows.reshape(NODE_W, 128).T

    # input shards
    evX = np.asarray(inputs['event_X'], np.float32)
    objX = np.asarray(inputs['object_X'], np.float32)
    evX_c = np.zeros((NC, EV_W * 128, D), np.float32)
    objX_c = np.zeros((NC, OBJ_PAD, D), np.float32)
    for c in range(NC):
        evX_c[c, :EV_SH] = evX[c * EV_SH:(c + 1) * EV_SH]
        objX_c[c, :OBJ_SH] = objX[c * OBJ_SH:(c + 1) * OBJ_SH]

    # replicated constant rows
    names = ['be', 'ge', 'ben', 'bo', 'go', 'bon', 'bu', 'g1', 'b1',
             'bl', 'g2', 'b2']
    vecs = [np.asarray(inputs[k], np.float32) for k in names]
    vecs.append(np.asarray(inputs['ah1'], np.float32)[D:])
    vecs.append(np.asarray(inputs['ah2'], np.float32)[D:])
    brep = np.tile(np.concatenate(vecs)[None, :], (128, 1)).astype(np.float32)
    meta['brep_names'] = names + ['a2_1', 'a2_2']

    def halves(w):
        w = np.asarray(w, np.float32)
        return w[:128, :].copy(), w[128:, :].copy()

    Wh1a = np.concatenate([np.asarray(inputs['Wh1'], np.float32),
                           (np.asarray(inputs['Wh1'], np.float32)
                            @ np.asarray(inputs['ah1'], np.float32)[:D])[:, None]], 1)
    Wh2a = np.concatenate([np.asarray(inputs['Wh2'], np.float32),
                           (np.asarray(inputs['Wh2'], np.float32)
                            @ np.asarray(inputs['ah2'], np.float32)[:D])[:, None]], 1)
    weights = {}
    for nm, w in [('wo', inputs['Wo']), ('we', inputs['We']),
                  ('wu', inputs['Wu']), ('wl', inputs['Wl']),
                  ('wh1', Wh1a), ('wh2', Wh2a)]:
        weights[nm + '0'], weights[nm + '1'] = halves(w)

    ident = np.eye(128, dtype=np.float32)
    iotaR = np.tile(np.arange(128, dtype=np.float32)[None, :], (128, 1))

    in_maps = []
    for c in range(NC):
        m = {
            'evX': evX_c[c], 'objX': objX_c[c],
            'oe_idx': oeP['idx'][c], 'oe_l': oeL[c],
            'hgA1_idx': hgAP['idx1'][c], 'hgA2_idx': hgAP['idx2'][c],
            'hgA_l': hgAL[c],
            'hgB_idx': hgBP['idx'][c], 'hgB_l': hgBL[c],
            'hgT1_idx': t1idx[c],
            'rcnt': rcnt_c[c],
            'brep': brep, 'ident': ident, 'iotaR': iotaR,
        }
        m.update(weights)
        in_maps.append(m)
    return meta, in_maps, origB


def _build(meta):
    nchO, nchA, nchB = meta['nchO'], meta['nchA'], meta['nchB']
    C_O, C_A, C_B = meta['C_O'], meta['C_A'], meta['C_B']

    nc = bacc.Bacc("TRN2", target_bir_lowering=False, debug=False,
                   enable_asserts=False, num_devices=NC)

    def din(name, shape, dt=F32):
        return nc.dram_tensor(name, shape, dt, kind="ExternalInput").ap()

    evX = din('evX', [EV_W * 128, D])
    objX = din('objX', [OBJ_PAD, D])
    oe_idx = din('oe_idx', [128, C_O], I32)
    oe_l = din('oe_l', [128, C_O])
    hgA1_idx = din('hgA1_idx', [128, C_A], I32)
    hgA2_idx = din('hgA2_idx', [128, C_A], I32)
    hgA_l = din('hgA_l', [128, C_A])
    hgB_idx = din('hgB_idx', [128, C_B], I32)
    hgB_l = din('hgB_l', [128, C_B])
    hgT1_idx = din('hgT1_idx', [128, NODE_W], I32)
    rcnt = din('rcnt', [EDGE_W * 128, 1])
    brep_i = din('brep', [128, 14 * D])
    ident_i = din('ident', [128, 128])
    iota_i = din('iotaR', [128, 128])
    wts = {nm: din(nm, [128, D]) for nm in
           ['wo0', 'wo1', 'we0', 'we1', 'wu0', 'wu1', 'wl0', 'wl1']}
    wts.update({nm: din(nm, [128, D + 1]) for nm in
                ['wh10', 'wh11', 'wh20', 'wh21']})

    h_out = nc.dram_tensor('h_out', [NODE_PAD, D], F32, kind="ExternalOutput").ap()
    attn_out = nc.dram_tensor('attn_out', [128, C_B], F32, kind="ExternalOutput").ap()

    BN = ['be', 'ge', 'ben', 'bo', 'go', 'bon', 'bu', 'g1', 'b1',
          'bl', 'g2', 'b2', 'a2_1', 'a2_2']

    from contextlib import ExitStack
    with tile.TileContext(nc) as tc, ExitStack() as es:
        consts = es.enter_context(tc.tile_pool(name="consts", bufs=1))
        dram = es.enter_context(tc.tile_pool(name="dram", bufs=1, space="DRAM"))
        io = es.enter_context(tc.tile_pool(name="io", bufs=4))
        trp = es.enter_context(tc.tile_pool(name="trp", bufs=6))
        gp = es.enter_context(tc.tile_pool(name="gp", bufs=4))
        sm = es.enter_context(tc.tile_pool(name="sm", bufs=8))
        ps_tr = es.enter_context(tc.tile_pool(name="ps_tr", bufs=2, space="PSUM"))
        ps_mm = es.enter_context(tc.tile_pool(name="ps_mm", bufs=3, space="PSUM"))
        ps_w = es.enter_context(tc.tile_pool(name="ps_w", bufs=2, space="PSUM"))

        # resident constants
        ident = consts.tile([128, 128], F32)
        nc.sync.dma_start(out=ident[:], in_=ident_i[:])
        iotaR = consts.tile([128, 128], F32)
        nc.sync.dma_start(out=iotaR[:], in_=iota_i[:])
        brep = consts.tile([128, 14 * D], F32)
        nc.sync.dma_start(out=brep[:], in_=brep_i[:])
        bv = {nm: brep[:, i * D:(i + 1) * D] for i, nm in enumerate(BN)}
        w = {}
        for nm, ap in wts.items():
            t = consts.tile([128, ap.shape[1]], F32, name=f"w_{nm}")
            nc.sync.dma_start(out=t[:], in_=ap[:])
            w[nm] = t
        eps = consts.tile([128, 1], F32)
        nc.vector.memset(eps[:], 1e-5)

        # DRAM intermediates
        evAG_in = dram.tile([EV_SH, D], F32)
        ev_full = dram.tile([N_EV, D], F32, addr_space="Shared")
        xw1_in = dram.tile([EV_SH + OBJ_PAD, D + 1], F32)
        xw1_full = dram.tile([XW1_ROWS, D + 1], F32, addr_space="Shared")
        obj_c = dram.tile([OBJ_PAD, D], F32)
        ef_in = dram.tile([EDGE_SH, D + 1], F32)
        ef_full = dram.tile([N_EV, D + 1], F32, addr_space="Shared")
        h1_c = dram.tile([NODE_PAD, D], F32)
        xw2_in = dram.tile([NODE_PAD, D + 1], F32)
        xw2_full = dram.tile([XW2_ROWS, D + 1], F32, addr_space="Shared")
        ef2_in = dram.tile([EDGE_SH, D + 1], F32)
        ef2_full = dram.tile([N_EV, D + 1], F32, addr_space="Shared")

        def lrelu_inplace(t, tmp_pool=sm):
            nc.vector.scalar_tensor_tensor(out=t[:], in0=t[:], scalar=0.2, in1=t[:],
                                           op0=OP.mult, op1=OP.max)

        def layernorm(y, gR, bR):
            stats = sm.tile([128, 6], F32, name="ln_stats", tag="ln_stats")
            nc.vector.bn_stats(out=stats[:], in_=y[:])
            mv = sm.tile([128, 2], F32, name="ln_mv", tag="ln_mv")
            nc.vector.bn_aggr(out=mv[:], in_=stats[:])
            std = sm.tile([128, 1], F32, name="ln_std", tag="ln_std")
            nc.scalar.activation(out=std[:], in_=mv[:, 1:2], func=AF.Sqrt, bias=eps[:])
            rstd = sm.tile([128, 1], F32, name="ln_rstd", tag="ln_rstd")
            nc.vector.reciprocal(out=rstd[:], in_=std[:])
            nc.vector.tensor_scalar(out=y[:], in0=y[:], scalar1=mv[:, 0:1],
                                    scalar2=rstd[:], op0=OP.subtract, op1=OP.mult)
            nc.vector.tensor_tensor(out=y[:], in0=y[:], in1=gR, op=OP.mult)
            nc.vector.tensor_tensor(out=y[:], in0=y[:], in1=bR, op=OP.add)

        def transpose2(x_tile):
            """[128,256] -> two [128,128] transposed tiles (d on partitions)."""
            pt = ps_tr.tile([128, 256], F32, name="trps", tag="tr")
            xts = []
            for k in range(2):
                nc.tensor.transpose(out=pt[:, 128 * k:128 * (k + 1)],
                                    in_=x_tile[:, 128 * k:128 * (k + 1)],
                                    identity=ident[:])
                st = trp.tile([128, 128], F32, name=f"trsb{k}", tag=f"trsb{k}")
                nc.vector.tensor_copy(out=st[:], in_=pt[:, 128 * k:128 * (k + 1)])
                xts.append(st)
            return xts

        def matmul_w(xts, w0, w1, ncols):
            pm = ps_w.tile([128, ncols], F32, name="mmw", tag="mmw")
            nc.tensor.matmul(out=pm[:], lhsT=xts[0][:], rhs=w0[:, :ncols],
                             start=True, stop=False)
            nc.tensor.matmul(out=pm[:], lhsT=xts[1][:], rhs=w1[:, :ncols],
                             start=False, stop=True)
            return pm

        def proj_block(x_tile, w0, w1, bR, gR, btR):
            """LN(lrelu(x@W + b)) * g + bt -> sbuf tile [128, D]"""
            xts = transpose2(x_tile)
            pm = matmul_w(xts, w0, w1, D)
            y = io.tile([128, D], F32, name="proj_y", tag="proj_y")
            nc.vector.tensor_tensor(out=y[:], in0=pm[:], in1=bR, op=OP.add)
            lrelu_inplace(y)
            layernorm(y, gR, btR)
            return y

        # ---------------- P1: event projection + evXW ----------------
        for wi in range(EV_W):
            rows = min(128, EV_SH - 128 * wi)
            xt = io.tile([128, D], F32, name="evx", tag="evx")
            nc.sync.dma_start(out=xt[:], in_=evX[128 * wi:128 * (wi + 1), :])
            ev_t = proj_block(xt, w['we0'], w['we1'], bv['be'], bv['ge'], bv['ben'])
            nc.sync.dma_start(out=evAG_in[128 * wi:128 * wi + rows, :],
                              in_=ev_t[:rows, :])
            xts = transpose2(ev_t)
            pm = matmul_w(xts, w['wh10'], w['wh11'], D + 1)
            xw_t = io.tile([128, D + 1], F32, name="xw_t", tag="xw_t")
            nc.vector.tensor_copy(out=xw_t[:], in_=pm[:])
            nc.sync.dma_start(out=xw1_in[128 * wi:128 * wi + rows, :],
                              in_=xw_t[:rows, :])

        # ---------------- P2: AllGather ev + evXW ----------------
        nc.gpsimd.collective_compute(
            "AllGather", OP.bypass, replica_groups=[list(range(NC))],
            ins=[evAG_in[:]], outs=[ev_full[:]])


        # ---------------- P3: object projection ----------------
        for wi in range(OBJ_W):
            xt = io.tile([128, D], F32, name="objx", tag="objx")
            nc.sync.dma_start(out=xt[:], in_=objX[128 * wi:128 * (wi + 1), :])
            ob_t = proj_block(xt, w['wo0'], w['wo1'], bv['bo'], bv['go'], bv['bon'])
            nc.sync.dma_start(out=obj_c[128 * wi:128 * (wi + 1), :], in_=ob_t[:])

        # ---------------- P4: oe segment sum + obj1/obj2 + objXW ----------------
        pos = 0
        for wi in range(OBJ_W):
            pmsg = ps_mm.tile([128, D], F32, name="pmsg", tag="acc")
            nch = int(nchO[wi])
            oix = sm.tile([128, meta['maxO']], I32, name="oe_ix", tag="oe_ix")
            nc.sync.dma_start(out=oix[:, :nch], in_=oe_idx[:, pos:pos + nch])
            olt = sm.tile([128, meta['maxO']], F32, name="oe_lw", tag="oe_lw")
            nc.sync.dma_start(out=olt[:, :nch], in_=oe_l[:, pos:pos + nch])
            for j in range(nch):
                g = gp.tile([128, D], F32, name="oe_g", tag="oe_g")
                nc.gpsimd.indirect_dma_start(
                    out=g[:], out_offset=None, in_=ev_full[:],
                    in_offset=bass.IndirectOffsetOnAxis(
                        ap=oix[:, j:j + 1], axis=0))
                P = trp.tile([128, 128], F32, name="oe_P", tag="oe_P")
                nc.vector.tensor_scalar(out=P[:], in0=iotaR[:], scalar1=olt[:, j:j + 1],
                                        scalar2=None, op0=OP.is_equal)
                nc.tensor.matmul(out=pmsg[:], lhsT=P[:], rhs=g[:],
                                 start=(j == 0), stop=(j == nch - 1))
            pos += nch
            # obj1 = LN(lrelu(msg@Wu + bu) + obj) * g1 + b1
            msg = io.tile([128, D], F32, name="msg", tag="msg")
            nc.vector.tensor_copy(out=msg[:], in_=pmsg[:])
            xts = transpose2(msg)
            pm1 = matmul_w(xts, w['wu0'], w['wu1'], D)
            y1 = io.tile([128, D], F32, name="y1", tag="y1")
            nc.vector.tensor_tensor(out=y1[:], in0=pm1[:], in1=bv['bu'], op=OP.add)
            lrelu_inplace(y1)
            ob = io.tile([128, D], F32, name="ob_in", tag="ob_in")
            nc.sync.dma_start(out=ob[:], in_=obj_c[128 * wi:128 * (wi + 1), :])
            nc.vector.tensor_tensor(out=y1[:], in0=y1[:], in1=ob[:], op=OP.add)
            layernorm(y1, bv['g1'], bv['b1'])
            # obj2 = LN(lrelu(obj1@Wl + bl) + obj1) * g2 + b2
            xts = transpose2(y1)
            pm2 = matmul_w(xts, w['wl0'], w['wl1'], D)
            y2 = io.tile([128, D], F32, name="y2", tag="y2")
            nc.vector.tensor_tensor(out=y2[:], in0=pm2[:], in1=bv['bl'], op=OP.add)
            lrelu_inplace(y2)
            nc.vector.tensor_tensor(out=y2[:], in0=y2[:], in1=y1[:], op=OP.add)
            layernorm(y2, bv['g2'], bv['b2'])
            # objXW
            xts = transpose2(y2)
            pm3 = matmul_w(xts, w['wh10'], w['wh11'], D + 1)
            xw_t = io.tile([128, D + 1], F32, name="oxw_t", tag="oxw_t")
            nc.vector.tensor_copy(out=xw_t[:], in_=pm3[:])
            nc.sync.dma_start(out=xw1_in[EV_SH + 128 * wi:EV_SH + 128 * (wi + 1), :],
                              in_=xw_t[:])

        # ---------------- P5: AllGather XW1 ----------------
        nc.gpsimd.collective_compute(
            "AllGather", OP.bypass, replica_groups=[list(range(NC))],
            ins=[xw1_in[:]], outs=[xw1_full[:]])

        def pass_a(idx_ap, src_full, ef_dst, a2R):
            pos = 0
            for wi in range(EDGE_W):
                rows = min(128, EDGE_SH - 128 * wi)
                pA = ps_mm.tile([128, D + 1], F32, name="pA", tag="acc")
                nch = int(nchA[wi])
                aix = sm.tile([128, meta['maxA']], I32, name="A_ix", tag="A_ix")
                nc.sync.dma_start(out=aix[:, :nch], in_=idx_ap[:, pos:pos + nch])
                alw = sm.tile([128, meta['maxA']], F32, name="A_lw", tag="A_lw")
                nc.sync.dma_start(out=alw[:, :nch], in_=hgA_l[:, pos:pos + nch])
                for j in range(nch):
                    g = gp.tile([128, D + 1], F32, name="A_g", tag="A_g")
                    nc.gpsimd.indirect_dma_start(
                        out=g[:], out_offset=None, in_=src_full[:],
                        in_offset=bass.IndirectOffsetOnAxis(
                            ap=aix[:, j:j + 1], axis=0))
                    P = trp.tile([128, 128], F32, name="A_P", tag="A_P")
                    nc.vector.tensor_scalar(out=P[:], in0=iotaR[:], scalar1=alw[:, j:j + 1],
                                            scalar2=None, op0=OP.is_equal)
                    nc.tensor.matmul(out=pA[:], lhsT=P[:], rhs=g[:],
                                     start=(j == 0), stop=(j == nch - 1))
                pos += nch
                rc = sm.tile([128, 1], F32, name="A_rc", tag="A_rc")
                nc.sync.dma_start(out=rc[:], in_=rcnt[128 * wi:128 * (wi + 1), :])
                ef_t = io.tile([128, D + 1], F32, name="ef_t", tag="ef_t")
                nc.vector.tensor_scalar_mul(out=ef_t[:, :D], in0=pA[:, :D],
                                            scalar1=rc[:])
                scr = trp.tile([128, D], F32, name="A_scr", tag="A_scr")
                nc.vector.tensor_tensor(out=scr[:], in0=ef_t[:, :D], in1=a2R,
                                        op=OP.mult)
                nc.vector.tensor_reduce(out=ef_t[:, D:D + 1], in_=scr[:],
                                        axis=mybir.AxisListType.X, op=OP.add)
                nc.sync.dma_start(out=ef_dst[128 * wi:128 * wi + rows, :],
                                  in_=ef_t[:rows, :])

        def pass_b(ef_src, t1_mode, xw_fuse, attn_tile):
            pos = 0
            maxB = meta['maxB']
            for wi in range(NODE_W):
                # t1 window column
                t1c = sm.tile([128, 1], F32, name="B_t1c", tag="B_t1c")
                if t1_mode == 'gather':
                    tix = sm.tile([128, 1], I32, name="B_tix", tag="B_tix")
                    nc.sync.dma_start(out=tix[:], in_=hgT1_idx[:, wi:wi + 1])
                    nc.gpsimd.indirect_dma_start(
                        out=t1c[:], out_offset=None, in_=xw1_full[:],
                        in_offset=bass.IndirectOffsetOnAxis(
                            ap=tix[:], axis=0),
                        element_offset=D)
                else:
                    nc.sync.dma_start(
                        out=t1c[:], in_=xw2_in[128 * wi:128 * (wi + 1), D:D + 1])
                ptr = ps_tr.tile([128, 128], F32, name="B_t1ps", tag="tr")
                nc.tensor.transpose(out=ptr[:], in_=t1c[:].to_broadcast([128, 128]),
                                    identity=ident[:])
                t1R = trp.tile([128, 128], F32, name="B_t1R", tag="B_t1R")
                nc.vector.tensor_copy(out=t1R[:], in_=ptr[:])

                nch = int(nchB[wi])
                bix = sm.tile([128, maxB], I32, name="B_ix", tag="B_ix")
                nc.sync.dma_start(out=bix[:, :nch], in_=hgB_idx[:, pos:pos + nch])
                blw = sm.tile([128, maxB], F32, name="B_lw", tag="B_lw")
                nc.sync.dma_start(out=blw[:, :nch], in_=hgB_l[:, pos:pos + nch])
                Ps, gths = [], []
                scores = sm.tile([128, maxB], F32, name="B_sc", tag="B_sc")
                for j in range(nch):
                    g = gp.tile([128, D + 1], F32, name=f"B_g{j}", tag=f"B_g{j}")
                    nc.gpsimd.indirect_dma_start(
                        out=g[:], out_offset=None, in_=ef_src[:],
                        in_offset=bass.IndirectOffsetOnAxis(
                            ap=bix[:, j:j + 1], axis=0))
                    gths.append(g)
                    P = trp.tile([128, 128], F32, name=f"B_P{j}", tag=f"B_P{j}")
                    nc.vector.tensor_scalar(out=P[:], in0=iotaR[:], scalar1=blw[:, j:j + 1],
                                            scalar2=None, op0=OP.is_equal)
                    Ps.append(P)
                    scr = trp.tile([128, 128], F32, name="B_scr", tag="B_scr")
                    nc.vector.tensor_tensor(out=scr[:], in0=P[:], in1=t1R[:],
                                            op=OP.mult)
                    t1i = sm.tile([128, 1], F32, name="B_t1i", tag="B_t1i")
                    nc.vector.tensor_reduce(out=t1i[:], in_=scr[:],
                                            axis=mybir.AxisListType.X, op=OP.add)
                    nc.vector.tensor_tensor(out=scores[:, j:j + 1], in0=t1i[:],
                                            in1=g[:, D:D + 1], op=OP.add)
                # lrelu + exp over the window's scores
                lrelu_inplace(scores[:, :nch])
                eT = sm.tile([128, maxB], F32, name="B_e", tag="B_e")
                nc.scalar.activation(out=eT[:, :nch], in_=scores[:, :nch], func=AF.Exp)
                pB = ps_mm.tile([128, D + 1], F32, name="pB", tag="acc")
                for j in range(nch):
                    rhs = gp.tile([128, D + 1], F32, name="B_rhs", tag="B_rhs")
                    nc.vector.tensor_scalar_mul(out=rhs[:, :D], in0=gths[j][:, :D],
                                                scalar1=eT[:, j:j + 1])
                    nc.vector.tensor_copy(out=rhs[:, D:D + 1], in_=eT[:, j:j + 1])
                    nc.tensor.matmul(out=pB[:], lhsT=Ps[j][:], rhs=rhs[:],
                                     start=(j == 0), stop=(j == nch - 1))
                # flush
                zc = sm.tile([128, 1], F32, name="B_zc", tag="B_zc")
                nc.vector.tensor_scalar(out=zc[:], in0=pB[:, D:D + 1], scalar1=1e-9,
                                        scalar2=None, op0=OP.max)
                rz = sm.tile([128, 1], F32, name="B_rz", tag="B_rz")
                nc.vector.reciprocal(out=rz[:], in_=zc[:])
                h_t = io.tile([128, D], F32, name="B_h", tag="B_h")
                nc.vector.tensor_scalar_mul(out=h_t[:], in0=pB[:, :D], scalar1=rz[:])
                lrelu_inplace(h_t)
                if xw_fuse:
                    xts = transpose2(h_t)
                    pmx = matmul_w(xts, w['wh20'], w['wh21'], D + 1)
                    xw_t = io.tile([128, D + 1], F32, name="B_xw", tag="B_xw")
                    nc.vector.tensor_copy(out=xw_t[:], in_=pmx[:])
                    nc.sync.dma_start(out=xw2_in[128 * wi:128 * (wi + 1), :],
                                      in_=xw_t[:])
                else:
                    nc.sync.dma_start(out=h_out[128 * wi:128 * (wi + 1), :],
                                      in_=h_t[:])
                    # attention per incidence: e * rz[node]
                    prz = ps_tr.tile([128, 128], F32, name="B_rzps", tag="tr")
                    nc.tensor.transpose(out=prz[:],
                                        in_=rz[:].to_broadcast([128, 128]),
                                        identity=ident[:])
                    rzR = trp.tile([128, 128], F32, name="B_rzR", tag="B_rzR")
                    nc.vector.tensor_copy(out=rzR[:], in_=prz[:])
                    for j in range(nch):
                        scr = trp.tile([128, 128], F32, name="B_scr2", tag="B_scr2")
                        nc.vector.tensor_tensor(out=scr[:], in0=Ps[j][:],
                                                in1=rzR[:], op=OP.mult)
                        rzi = sm.tile([128, 1], F32, name="B_rzi", tag="B_rzi")
                        nc.vector.tensor_reduce(out=rzi[:], in_=scr[:],
                                                axis=mybir.AxisListType.X, op=OP.add)
                        nc.vector.tensor_tensor(out=attn_tile[:, pos + j:pos + j + 1],
                                                in0=rzi[:], in1=eT[:, j:j + 1],
                                                op=OP.mult)
                pos += nch

        a2_1 = bv['a2_1']
        a2_2 = bv['a2_2']
        attn_t = consts.tile([128, C_B], F32)

        # ---------------- Layer 1 ----------------
        pass_a(hgA1_idx, xw1_full, ef_in, a2_1)
        nc.gpsimd.collective_compute(
            "AllGather", OP.bypass, replica_groups=[list(range(NC))],
            ins=[ef_in[:]], outs=[ef_full[:]])
        pass_b(ef_full, 'gather', True, attn_t)
        nc.gpsimd.collective_compute(
            "AllGather", OP.bypass, replica_groups=[list(range(NC))],
            ins=[xw2_in[:]], outs=[xw2_full[:]])

        # ---------------- Layer 2 ----------------
        pass_a(hgA2_idx, xw2_full, ef2_in, a2_2)
        nc.gpsimd.collective_compute(
            "AllGather", OP.bypass, replica_groups=[list(range(NC))],
            ins=[ef2_in[:]], outs=[ef2_full[:]])
        pass_b(ef2_full, 'local', False, attn_t)

        nc.sync.dma_start(out=attn_out[:], in_=attn_t[:])

    nc.compile()
    return nc


def kernel(**inputs):
    meta, in_maps, origB = _prep(inputs)
    nc = _build(meta)
    res = bass_utils.run_bass_kernel_spmd(nc, in_maps, core_ids=list(range(NC)))
    h = np.zeros((N_NODE, D), np.float32)
    attn = np.zeros((E2,), np.float32)
    for c in range(NC):
        h[c * NODE_SH:(c + 1) * NODE_SH] = res.results[c]['h_out'][:NODE_SH]
        a = res.results[c]['attn_out']
        m = origB[c]
        valid = m >= 0
        attn[m[valid]] = a[valid]
    return h, attn


# revision 12
# speedup vs baseline: 1.4249x; 1.0702x over previous
"""Trainium2 Bass kernel for nn_Conv_hg_90022514524500 (hypergraph GNN).

Self-contained: hardcodes problem shapes, shards across 8 NeuronCores,
runs one SPMD Bass/Tile program via bass_utils.run_bass_kernel_spmd.
"""
import numpy as np

import concourse.bass as bass
import concourse.bacc as bacc
import concourse.tile as tile
import concourse.mybir as mybir
from concourse import bass_utils

F32 = mybir.dt.float32
I32 = mybir.dt.int32
AF = mybir.ActivationFunctionType
OP = mybir.AluOpType

N_EV, N_OBJ, D = 50000, 100000, 256
E1, E2 = 400000, 800000
NC = 8
EV_SH = N_EV // NC            # 6250
OBJ_SH = N_OBJ // NC          # 12500
NODE_SH = (N_EV + N_OBJ) // NC  # 18750
EDGE_SH = N_EV // NC          # 6250
EV_W = (EV_SH + 127) // 128       # 49
OBJ_W = (OBJ_SH + 127) // 128     # 98
NODE_W = (NODE_SH + 127) // 128   # 147
EDGE_W = (EDGE_SH + 127) // 128   # 49
OBJ_PAD = OBJ_W * 128             # 12544
NODE_PAD = NODE_W * 128           # 18816
PAD_L = 200.0                     # one-hot "no segment" sentinel

N_NODE = N_EV + N_OBJ
XW1_ROWS = NC * (N_EV // NC + (N_OBJ // NC + 127) // 128 * 128)  # 150352
XW2_ROWS = NC * NODE_PAD          # 150528: padded node rows


XW1_CH = EV_SH + OBJ_PAD   # 18794 rows per rank in XW1_full


def _xw1_row(node):
    """Row of node's features in XW1_full (per-rank [evXW | objXW] chunks)."""
    node = np.asarray(node)
    obj = node - N_EV
    return np.where(node < N_EV,
                    XW1_CH * (node // EV_SH) + node % EV_SH,
                    XW1_CH * (obj // OBJ_SH) + EV_SH + obj % OBJ_SH).astype(np.int64)


def _xw2_row(node):
    node = np.asarray(node)
    return (NODE_PAD * (node // NODE_SH) + node % NODE_SH).astype(np.int64)


def _build_streams(seg, n_seg_per_core, n_win, payload_cols):
    """Group incidences by core (seg // n_seg_per_core), sort by seg, pack into
    SPMD-uniform (128 x C) slot arrays chunked per 128-segment window.

    payload_cols: dict name -> per-incidence int64/float array (original order).
    Returns nch[n_win], C, dict name -> [NC,128,C] arrays, l [NC,128,C] f32,
    orig [NC,128,C] int64 (original incidence idx, -1 pads).
    """
    n = seg.shape[0]
    core = seg // n_seg_per_core
    local = seg - core * n_seg_per_core
    win = local // 128
    # counts[c, w]
    counts = np.zeros((NC, n_win), np.int64)
    np.add.at(counts, (core, win), 1)
    nch = np.maximum(1, (counts.max(0) + 127) // 128)  # per window, shared by cores
    base = np.concatenate([[0], np.cumsum(nch)])       # chunk base per window
    C = int(base[-1])

    out = {k: np.zeros((NC, 128, C), v.dtype) for k, v in payload_cols.items()}
    l_arr = np.full((NC, 128, C), PAD_L, np.float32)
    orig = np.full((NC, 128, C), -1, np.int64)

    order = np.lexsort((local, core))   # by core, then by local seg (stable)
    core_s = core[order]
    local_s = local[order]
    win_s = win[order]
    # position within (core, window): running index over sorted groups
    key = core_s * n_win + win_s
    start_of_group = np.concatenate([[True], key[1:] != key[:-1]])
    group_id = np.cumsum(start_of_group) - 1
    group_first = np.where(start_of_group)[0]
    pwin = np.arange(n) - group_first[group_id]       # 0..count-1 within group
    col = base[win_s] + pwin // 128
    p = pwin % 128
    l_arr[core_s, p, col] = (local_s - 128 * win_s).astype(np.float32)
    orig[core_s, p, col] = order
    for k, v in payload_cols.items():
        out[k][core_s, p, col] = v[order]
    return nch, C, out, l_arr, orig


def _prep(inputs):
    """All host-side preprocessing. Returns (meta, per-core input maps pieces)."""
    oe_ev = np.asarray(inputs['oe_ev']).astype(np.int64)
    oe_obj = np.asarray(inputs['oe_obj']).astype(np.int64)
    hg_node = np.asarray(inputs['hg_node']).astype(np.int64)
    hg_edge = np.asarray(inputs['hg_edge']).astype(np.int64)

    meta = {}
    # oe stream: segments = objects
    nchO, C_O, oeP, oeL, _ = _build_streams(
        oe_obj, OBJ_SH, OBJ_W, {'idx': oe_ev.astype(np.int32)})
    # pass A stream: segments = edges; payload = node (converted per layer)
    nchA, C_A, hgAP, hgAL, _ = _build_streams(
        hg_edge, EDGE_SH, EDGE_W,
        {'idx1': _xw1_row(hg_node).astype(np.int32),
         'idx2': _xw2_row(hg_node).astype(np.int32)})
    # pass B stream: segments = nodes; payload = edge (ef row)
    nchB, C_B, hgBP, hgBL, origB = _build_streams(
        hg_node, NODE_SH, NODE_W, {'idx': hg_edge.astype(np.int32)})

    meta['nchO'], meta['C_O'] = nchO, C_O
    meta['nchA'], meta['C_A'] = nchA, C_A
    meta['nchB'], meta['C_B'] = nchB, C_B
    meta['maxB'] = int(nchB.max())
    meta['maxA'] = int(nchA.max())
    meta['maxO'] = int(nchO.max())

    # edge mean reciprocal counts, padded per core [EDGE_W*128]
    cnt = np.bincount(hg_edge, minlength=N_EV).astype(np.float32)
    rcnt = 1.0 / np.maximum(cnt, 1.0)
    rcnt_c = np.ones((NC, EDGE_W * 128, 1), np.float32)
    for c in range(NC):
        rcnt_c[c, :EDGE_SH, 0] = rcnt[c * EDGE_SH:(c + 1) * EDGE_SH]

    # t1 gather rows for L1 pass B windows: [NC, 128, NODE_W]
    t1idx = np.zeros((NC, 128, NODE_W), np.int32)
    for c in range(NC):
        ln = np.arange(NODE_PAD)
        node = c * NODE_SH + np.minimum(ln, NODE_SH - 1)
        rows = _xw1_row(node).astype(np.int32)
        t1idx[c] = rows.reshape(NODE_W, 128).T

    # input shards
    evX = np.asarray(inputs['event_X'], np.float32)
    objX = np.asarray(inputs['object_X'], np.float32)
    evX_c = np.zeros((NC, EV_W * 128, D), np.float32)
    objX_c = np.zeros((NC, OBJ_PAD, D), np.float32)
    for c in range(NC):
        evX_c[c, :EV_SH] = evX[c * EV_SH:(c + 1) * EV_SH]
        objX_c[c, :OBJ_SH] = objX[c * OBJ_SH:(c + 1) * OBJ_SH]

    # replicated constant rows
    names = ['be', 'ge', 'ben', 'bo', 'go', 'bon', 'bu', 'g1', 'b1',
             'bl', 'g2', 'b2']
    vecs = [np.asarray(inputs[k], np.float32) for k in names]
    vecs.append(np.asarray(inputs['ah1'], np.float32)[D:])
    vecs.append(np.asarray(inputs['ah2'], np.float32)[D:])
    brep = np.tile(np.concatenate(vecs)[None, :], (128, 1)).astype(np.float32)
    meta['brep_names'] = names + ['a2_1', 'a2_2']

    def halves(w):
        w = np.asarray(w, np.float32)
        return w[:128, :].copy(), w[128:, :].copy()

    Wh1a = np.concatenate([np.asarray(inputs['Wh1'], np.float32),
                           (np.asarray(inputs['Wh1'], np.float32)
                            @ np.asarray(inputs['ah1'], np.float32)[:D])[:, None]], 1)
    Wh2a = np.concatenate([np.asarray(inputs['Wh2'], np.float32),
                           (np.asarray(inputs['Wh2'], np.float32)
                            @ np.asarray(inputs['ah2'], np.float32)[:D])[:, None]], 1)
    weights = {}
    for nm, w in [('wo', inputs['Wo']), ('we', inputs['We']),
                  ('wu', inputs['Wu']), ('wl', inputs['Wl']),
                  ('wh1', Wh1a), ('wh2', Wh2a)]:
        weights[nm + '0'], weights[nm + '1'] = halves(w)

    ident = np.eye(128, dtype=np.float32)
    iotaR = np.tile(np.arange(128, dtype=np.float32)[None, :], (128, 1))

    in_maps = []
    for c in range(NC):
        m = {
            'evX': evX_c[c], 'objX': objX_c[c],
            'oe_idx': oeP['idx'][c], 'oe_l': oeL[c],
            'hgA1_idx': hgAP['idx1'][c], 'hgA2_idx': hgAP['idx2'][c],
            'hgA_l': hgAL[c],
            'hgB_idx': hgBP['idx'][c], 'hgB_l': hgBL[c],
            'hgT1_idx': t1idx[c],
            'rcnt': rcnt_c[c],
            'brep': brep, 'ident': ident, 'iotaR': iotaR,
        }
        m.update(weights)
        in_maps.append(m)
    return meta, in_maps, origB


def _build(meta):
    nchO, nchA, nchB = meta['nchO'], meta['nchA'], meta['nchB']
    C_O, C_A, C_B = meta['C_O'], meta['C_A'], meta['C_B']

    nc = bacc.Bacc("TRN2", target_bir_lowering=False, debug=False,
                   enable_asserts=False, num_devices=NC)

    def din(name, shape, dt=F32):
        return nc.dram_tensor(name, shape, dt, kind="ExternalInput").ap()

    evX = din('evX', [EV_W * 128, D])
    objX = din('objX', [OBJ_PAD, D])
    oe_idx = din('oe_idx', [128, C_O], I32)
    oe_l = din('oe_l', [128, C_O])
    hgA1_idx = din('hgA1_idx', [128, C_A], I32)
    hgA2_idx = din('hgA2_idx', [128, C_A], I32)
    hgA_l = din('hgA_l', [128, C_A])
    hgB_idx = din('hgB_idx', [128, C_B], I32)
    hgB_l = din('hgB_l', [128, C_B])
    hgT1_idx = din('hgT1_idx', [128, NODE_W], I32)
    rcnt = din('rcnt', [EDGE_W * 128, 1])
    brep_i = din('brep', [128, 14 * D])
    ident_i = din('ident', [128, 128])
    iota_i = din('iotaR', [128, 128])
    wts = {nm: din(nm, [128, D]) for nm in
           ['wo0', 'wo1', 'we0', 'we1', 'wu0', 'wu1', 'wl0', 'wl1']}
    wts.update({nm: din(nm, [128, D + 1]) for nm in
                ['wh10', 'wh11', 'wh20', 'wh21']})

    h_out = nc.dram_tensor('h_out', [NODE_PAD, D], F32, kind="ExternalOutput").ap()
    attn_out = nc.dram_tensor('attn_out', [128, C_B], F32, kind="ExternalOutput").ap()

    BN = ['be', 'ge', 'ben', 'bo', 'go', 'bon', 'bu', 'g1', 'b1',
          'bl', 'g2', 'b2', 'a2_1', 'a2_2']

    from contextlib import ExitStack
    with tile.TileContext(nc) as tc, ExitStack() as es:
        consts = es.enter_context(tc.tile_pool(name="consts", bufs=1))
        dram = es.enter_context(tc.tile_pool(name="dram", bufs=1, space="DRAM"))
        io = es.enter_context(tc.tile_pool(name="io", bufs=4))
        trp = es.enter_context(tc.tile_pool(name="trp", bufs=6))
        gp = es.enter_context(tc.tile_pool(name="gp", bufs=4))
        sm = es.enter_context(tc.tile_pool(name="sm", bufs=8))
        ps_tr = es.enter_context(tc.tile_pool(name="ps_tr", bufs=2, space="PSUM"))
        ps_mm = es.enter_context(tc.tile_pool(name="ps_mm", bufs=3, space="PSUM"))
        ps_w = es.enter_context(tc.tile_pool(name="ps_w", bufs=3, space="PSUM"))

        # resident constants
        ident = consts.tile([128, 128], F32)
        nc.sync.dma_start(out=ident[:], in_=ident_i[:])
        iotaR = consts.tile([128, 128], F32)
        nc.sync.dma_start(out=iotaR[:], in_=iota_i[:])
        brep = consts.tile([128, 14 * D], F32)
        nc.sync.dma_start(out=brep[:], in_=brep_i[:])
        bv = {nm: brep[:, i * D:(i + 1) * D] for i, nm in enumerate(BN)}
        w = {}
        for nm, ap in wts.items():
            t = consts.tile([128, ap.shape[1]], F32, name=f"w_{nm}")
            nc.sync.dma_start(out=t[:], in_=ap[:])
            w[nm] = t
        eps = consts.tile([128, 1], F32)
        nc.vector.memset(eps[:], 1e-5)

        # DRAM intermediates
        evAG_in = dram.tile([EV_SH, D], F32)
        ev_full = dram.tile([N_EV, D], F32, addr_space="Shared")
        xw1_in = dram.tile([EV_SH + OBJ_PAD, D + 1], F32)
        xw1_full = dram.tile([XW1_ROWS, D + 1], F32, addr_space="Shared")
        obj_c = dram.tile([OBJ_PAD, D], F32)
        ef_in = dram.tile([EDGE_SH, D + 1], F32)
        ef_full = dram.tile([N_EV, D + 1], F32, addr_space="Shared")
        h1_c = dram.tile([NODE_PAD, D], F32)
        xw2_in = dram.tile([NODE_PAD, D + 1], F32)
        xw2_full = dram.tile([XW2_ROWS, D + 1], F32, addr_space="Shared")
        ef2_in = dram.tile([EDGE_SH, D + 1], F32)
        ef2_full = dram.tile([N_EV, D + 1], F32, addr_space="Shared")

        def lrelu_inplace(t, tmp_pool=sm):
            nc.vector.scalar_tensor_tensor(out=t[:], in0=t[:], scalar=0.2, in1=t[:],
                                           op0=OP.mult, op1=OP.max)

        def layernorm(y, gR, bR):
            stats = sm.tile([128, 6], F32, name="ln_stats", tag="ln_stats")
            nc.vector.bn_stats(out=stats[:], in_=y[:])
            mv = sm.tile([128, 2], F32, name="ln_mv", tag="ln_mv")
            nc.vector.bn_aggr(out=mv[:], in_=stats[:])
            std = sm.tile([128, 1], F32, name="ln_std", tag="ln_std")
            nc.scalar.activation(out=std[:], in_=mv[:, 1:2], func=AF.Sqrt, bias=eps[:])
            rstd = sm.tile([128, 1], F32, name="ln_rstd", tag="ln_rstd")
            nc.vector.reciprocal(out=rstd[:], in_=std[:])
            nc.vector.tensor_scalar(out=y[:], in0=y[:], scalar1=mv[:, 0:1],
                                    scalar2=rstd[:], op0=OP.subtract, op1=OP.mult)
            nc.vector.tensor_tensor(out=y[:], in0=y[:], in1=gR, op=OP.mult)
            nc.vector.tensor_tensor(out=y[:], in0=y[:], in1=bR, op=OP.add)

        def transpose2(x_tile):
            """[128,256] -> two [128,128] transposed tiles (d on partitions)."""
            pt = ps_tr.tile([128, 256], F32, name="trps", tag="tr")
            xts = []
            for k in range(2):
                nc.tensor.transpose(out=pt[:, 128 * k:128 * (k + 1)],
                                    in_=x_tile[:, 128 * k:128 * (k + 1)],
                                    identity=ident[:])
                st = trp.tile([128, 128], F32, name=f"trsb{k}", tag=f"trsb{k}")
                nc.vector.tensor_copy(out=st[:], in_=pt[:, 128 * k:128 * (k + 1)])
                xts.append(st)
            return xts

        def matmul_w(xts, w0, w1, ncols):
            pm = ps_w.tile([128, ncols], F32, name="mmw", tag="mmw")
            nc.tensor.matmul(out=pm[:], lhsT=xts[0][:], rhs=w0[:, :ncols],
                             start=True, stop=False)
            nc.tensor.matmul(out=pm[:], lhsT=xts[1][:], rhs=w1[:, :ncols],
                             start=False, stop=True)
            return pm

        def proj_block(x_tile, w0, w1, bR, gR, btR):
            """LN(lrelu(x@W + b)) * g + bt -> sbuf tile [128, D]"""
            xts = transpose2(x_tile)
            pm = matmul_w(xts, w0, w1, D)
            y = io.tile([128, D], F32, name="proj_y", tag="proj_y")
            nc.vector.tensor_tensor(out=y[:], in0=pm[:], in1=bR, op=OP.add)
            lrelu_inplace(y)
            layernorm(y, gR, btR)
            return y

        # ---------------- P1: event projection + evXW ----------------
        for wi in range(EV_W):
            rows = min(128, EV_SH - 128 * wi)
            xt = io.tile([128, D], F32, name="evx", tag="evx")
            nc.sync.dma_start(out=xt[:], in_=evX[128 * wi:128 * (wi + 1), :])
            ev_t = proj_block(xt, w['we0'], w['we1'], bv['be'], bv['ge'], bv['ben'])
            nc.sync.dma_start(out=evAG_in[128 * wi:128 * wi + rows, :],
                              in_=ev_t[:rows, :])
            xts = transpose2(ev_t)
            pm = matmul_w(xts, w['wh10'], w['wh11'], D + 1)
            xw_t = io.tile([128, D + 1], F32, name="xw_t", tag="xw_t")
            nc.vector.tensor_copy(out=xw_t[:], in_=pm[:])
            nc.sync.dma_start(out=xw1_in[128 * wi:128 * wi + rows, :],
                              in_=xw_t[:rows, :])

        # ---------------- P2: AllGather ev + evXW ----------------
        nc.gpsimd.collective_compute(
            "AllGather", OP.bypass, replica_groups=[list(range(NC))],
            ins=[evAG_in[:]], outs=[ev_full[:]])


        # ---------------- P3: object projection ----------------
        for wi in range(OBJ_W):
            xt = io.tile([128, D], F32, name="objx", tag="objx")
            nc.sync.dma_start(out=xt[:], in_=objX[128 * wi:128 * (wi + 1), :])
            ob_t = proj_block(xt, w['wo0'], w['wo1'], bv['bo'], bv['go'], bv['bon'])
            nc.sync.dma_start(out=obj_c[128 * wi:128 * (wi + 1), :], in_=ob_t[:])

        # ---------------- P4: oe segment sum + obj1/obj2 + objXW ----------------
        pos = 0
        for wi in range(OBJ_W):
            pmsg = ps_mm.tile([128, D], F32, name="pmsg", tag="acc")
            nch = int(nchO[wi])
            oix = sm.tile([128, meta['maxO']], I32, name="oe_ix", tag="oe_ix")
            nc.sync.dma_start(out=oix[:, :nch], in_=oe_idx[:, pos:pos + nch])
            olt = sm.tile([128, meta['maxO']], F32, name="oe_lw", tag="oe_lw")
            nc.sync.dma_start(out=olt[:, :nch], in_=oe_l[:, pos:pos + nch])
            for j in range(nch):
                g = gp.tile([128, D], F32, name="oe_g", tag="oe_g")
                nc.gpsimd.indirect_dma_start(
                    out=g[:], out_offset=None, in_=ev_full[:],
                    in_offset=bass.IndirectOffsetOnAxis(
                        ap=oix[:, j:j + 1], axis=0))
                P = trp.tile([128, 128], F32, name="oe_P", tag="oe_P")
                nc.vector.tensor_scalar(out=P[:], in0=iotaR[:], scalar1=olt[:, j:j + 1],
                                        scalar2=None, op0=OP.is_equal)
                nc.tensor.matmul(out=pmsg[:], lhsT=P[:], rhs=g[:],
                                 start=(j == 0), stop=(j == nch - 1))
            pos += nch
            # obj1 = LN(lrelu(msg@Wu + bu) + obj) * g1 + b1
            msg = io.tile([128, D], F32, name="msg", tag="msg")
            nc.vector.tensor_copy(out=msg[:], in_=pmsg[:])
            xts = transpose2(msg)
            pm1 = matmul_w(xts, w['wu0'], w['wu1'], D)
            y1 = io.tile([128, D], F32, name="y1", tag="y1")
            nc.vector.tensor_tensor(out=y1[:], in0=pm1[:], in1=bv['bu'], op=OP.add)
            lrelu_inplace(y1)
            ob = io.tile([128, D], F32, name="ob_in", tag="ob_in")
            nc.sync.dma_start(out=ob[:], in_=obj_c[128 * wi:128 * (wi + 1), :])
            nc.vector.tensor_tensor(out=y1[:], in0=y1[:], in1=ob[:], op=OP.add)
            layernorm(y1, bv['g1'], bv['b1'])
            # obj2 = LN(lrelu(obj1@Wl + bl) + obj1) * g2 + b2
            xts = transpose2(y1)
            pm2 = matmul_w(xts, w['wl0'], w['wl1'], D)
            y2 = io.tile([128, D], F32, name="y2", tag="y2")
            nc.vector.tensor_tensor(out=y2[:], in0=pm2[:], in1=bv['bl'], op=OP.add)
            lrelu_inplace(y2)
            nc.vector.tensor_tensor(out=y2[:], in0=y2[:], in1=y1[:], op=OP.add)
            layernorm(y2, bv['g2'], bv['b2'])
            # objXW
            xts = transpose2(y2)
            pm3 = matmul_w(xts, w['wh10'], w['wh11'], D + 1)
            xw_t = io.tile([128, D + 1], F32, name="oxw_t", tag="oxw_t")
            nc.vector.tensor_copy(out=xw_t[:], in_=pm3[:])
            nc.sync.dma_start(out=xw1_in[EV_SH + 128 * wi:EV_SH + 128 * (wi + 1), :],
                              in_=xw_t[:])

        # ---------------- P5: AllGather XW1 ----------------
        nc.gpsimd.collective_compute(
            "AllGather", OP.bypass, replica_groups=[list(range(NC))],
            ins=[xw1_in[:]], outs=[xw1_full[:]])

        def pass_a(idx_ap, src_full, ef_dst, a2R):
            pos = 0
            for wi in range(EDGE_W):
                rows = min(128, EDGE_SH - 128 * wi)
                pA = ps_mm.tile([128, D + 1], F32, name="pA", tag="acc")
                nch = int(nchA[wi])
                aix = sm.tile([128, meta['maxA']], I32, name="A_ix", tag="A_ix")
                nc.sync.dma_start(out=aix[:, :nch], in_=idx_ap[:, pos:pos + nch])
                alw = sm.tile([128, meta['maxA']], F32, name="A_lw", tag="A_lw")
                nc.sync.dma_start(out=alw[:, :nch], in_=hgA_l[:, pos:pos + nch])
                for j in range(nch):
                    g = gp.tile([128, D + 1], F32, name="A_g", tag="A_g")
                    nc.gpsimd.indirect_dma_start(
                        out=g[:], out_offset=None, in_=src_full[:],
                        in_offset=bass.IndirectOffsetOnAxis(
                            ap=aix[:, j:j + 1], axis=0))
                    P = trp.tile([128, 128], F32, name="A_P", tag="A_P")
                    nc.vector.tensor_scalar(out=P[:], in0=iotaR[:], scalar1=alw[:, j:j + 1],
                                            scalar2=None, op0=OP.is_equal)
                    nc.tensor.matmul(out=pA[:], lhsT=P[:], rhs=g[:],
                                     start=(j == 0), stop=(j == nch - 1))
                pos += nch
                rc = sm.tile([128, 1], F32, name="A_rc", tag="A_rc")
                nc.sync.dma_start(out=rc[:], in_=rcnt[128 * wi:128 * (wi + 1), :])
                ef_t = io.tile([128, D + 1], F32, name="ef_t", tag="ef_t")
                nc.vector.tensor_scalar_mul(out=ef_t[:, :D], in0=pA[:, :D],
                                            scalar1=rc[:])
                scr = trp.tile([128, D], F32, name="A_scr", tag="A_scr")
                nc.vector.tensor_tensor(out=scr[:], in0=ef_t[:, :D], in1=a2R,
                                        op=OP.mult)
                nc.vector.tensor_reduce(out=ef_t[:, D:D + 1], in_=scr[:],
                                        axis=mybir.AxisListType.X, op=OP.add)
                nc.sync.dma_start(out=ef_dst[128 * wi:128 * wi + rows, :],
                                  in_=ef_t[:rows, :])

        def pass_b(ef_src, t1_mode, xw_fuse, attn_tile):
            pos = 0
            maxB = meta['maxB']
            for wi in range(NODE_W):
                # t1 window column
                t1c = sm.tile([128, 1], F32, name="B_t1c", tag="B_t1c")
                if t1_mode == 'gather':
                    tix = sm.tile([128, 1], I32, name="B_tix", tag="B_tix")
                    nc.sync.dma_start(out=tix[:], in_=hgT1_idx[:, wi:wi + 1])
                    nc.gpsimd.indirect_dma_start(
                        out=t1c[:], out_offset=None, in_=xw1_full[:],
                        in_offset=bass.IndirectOffsetOnAxis(
                            ap=tix[:], axis=0),
                        element_offset=D)
                else:
                    nc.sync.dma_start(
                        out=t1c[:], in_=xw2_in[128 * wi:128 * (wi + 1), D:D + 1])
                ptr = ps_tr.tile([128, 128], F32, name="B_t1ps", tag="tr")
                nc.tensor.transpose(out=ptr[:], in_=t1c[:].to_broadcast([128, 128]),
                                    identity=ident[:])
                t1R = trp.tile([128, 128], F32, name="B_t1R", tag="B_t1R")
                nc.vector.tensor_copy(out=t1R[:], in_=ptr[:])

                nch = int(nchB[wi])
                bix = sm.tile([128, maxB], I32, name="B_ix", tag="B_ix")
                nc.sync.dma_start(out=bix[:, :nch], in_=hgB_idx[:, pos:pos + nch])
                blw = sm.tile([128, maxB], F32, name="B_lw", tag="B_lw")
                nc.sync.dma_start(out=blw[:, :nch], in_=hgB_l[:, pos:pos + nch])
                Ps, gths = [], []
                scores = sm.tile([128, maxB], F32, name="B_sc", tag="B_sc")
                for j in range(nch):
                    g = gp.tile([128, D + 1], F32, name=f"B_g{j}", tag=f"B_g{j}")
                    nc.gpsimd.indirect_dma_start(
                        out=g[:], out_offset=None, in_=ef_src[:],
                        in_offset=bass.IndirectOffsetOnAxis(
                            ap=bix[:, j:j + 1], axis=0))
                    gths.append(g)
                    P = trp.tile([128, 128], F32, name=f"B_P{j}", tag=f"B_P{j}")
                    nc.vector.tensor_scalar(out=P[:], in0=iotaR[:], scalar1=blw[:, j:j + 1],
                                            scalar2=None, op0=OP.is_equal)
                    Ps.append(P)
                    scr = trp.tile([128, 128], F32, name="B_scr", tag="B_scr")
                    nc.vector.tensor_tensor(out=scr[:], in0=P[:], in1=t1R[:],
                                            op=OP.mult)
                    t1i = sm.tile([128, 1], F32, name="B_t1i", tag="B_t1i")
                    nc.vector.tensor_reduce(out=t1i[:], in_=scr[:],
                                            axis=mybir.AxisListType.X, op=OP.add)
                    nc.vector.tensor_tensor(out=scores[:, j:j + 1], in0=t1i[:],
                                            in1=g[:, D:D + 1], op=OP.add)
                # lrelu + exp over the window's scores
                lrelu_inplace(scores[:, :nch])
                eT = sm.tile([128, maxB], F32, name="B_e", tag="B_e")
                nc.scalar.activation(out=eT[:, :nch], in_=scores[:, :nch], func=AF.Exp)
                pB = ps_mm.tile([128, D + 1], F32, name="pB", tag="acc")
                for j in range(nch):
                    rhs = gp.tile([128, D + 1], F32, name="B_rhs", tag="B_rhs")
                    nc.vector.tensor_scalar_mul(out=rhs[:, :D], in0=gths[j][:, :D],
                                                scalar1=eT[:, j:j + 1])
                    nc.vector.tensor_copy(out=rhs[:, D:D + 1], in_=eT[:, j:j + 1])
                    nc.tensor.matmul(out=pB[:], lhsT=Ps[j][:], rhs=rhs[:],
                                     start=(j == 0), stop=(j == nch - 1))
                # flush
                zc = sm.tile([128, 1], F32, name="B_zc", tag="B_zc")
                nc.vector.tensor_scalar(out=zc[:], in0=pB[:, D:D + 1], scalar1=1e-9,
                                        scalar2=None, op0=OP.max)
                rz = sm.tile([128, 1], F32, name="B_rz", tag="B_rz")
                nc.vector.reciprocal(out=rz[:], in_=zc[:])
                h_t = io.tile([128, D], F32, name="B_h", tag="B_h")
                nc.vector.tensor_scalar_mul(out=h_t[:], in0=pB[:, :D], scalar1=rz[:])
                lrelu_inplace(h_t)
                if xw_fuse:
                    xts = transpose2(h_t)
                    pmx = matmul_w(xts, w['wh20'], w['wh21'], D + 1)
                    xw_t = io.tile([128, D + 1], F32, name="B_xw", tag="B_xw")
                    nc.vector.tensor_copy(out=xw_t[:], in_=pmx[:])
                    nc.sync.dma_start(out=xw2_in[128 * wi:128 * (wi + 1), :],
                                      in_=xw_t[:])
                else:
                    nc.sync.dma_start(out=h_out[128 * wi:128 * (wi + 1), :],
                                      in_=h_t[:])
                    # attention per incidence: e * rz[node]
                    prz = ps_tr.tile([128, 128], F32, name="B_rzps", tag="tr")
                    nc.tensor.transpose(out=prz[:],
                                        in_=rz[:].to_broadcast([128, 128]),
                                        identity=ident[:])
                    rzR = trp.tile([128, 128], F32, name="B_rzR", tag="B_rzR")
                    nc.vector.tensor_copy(out=rzR[:], in_=prz[:])
                    for j in range(nch):
                        scr = trp.tile([128, 128], F32, name="B_scr2", tag="B_scr2")
                        nc.vector.tensor_tensor(out=scr[:], in0=Ps[j][:],
                                                in1=rzR[:], op=OP.mult)
                        rzi = sm.tile([128, 1], F32, name="B_rzi", tag="B_rzi")
                        nc.vector.tensor_reduce(out=rzi[:], in_=scr[:],
                                                axis=mybir.AxisListType.X, op=OP.add)
                        nc.vector.tensor_tensor(out=attn_tile[:, pos + j:pos + j + 1],
                                                in0=rzi[:], in1=eT[:, j:j + 1],
                                                op=OP.mult)
                pos += nch

        a2_1 = bv['a2_1']
        a2_2 = bv['a2_2']
        attn_t = consts.tile([128, C_B], F32)

        # ---------------- Layer 1 ----------------
        pass_a(hgA1_idx, xw1_full, ef_in, a2_1)
        nc.gpsimd.collective_compute(
            "AllGather", OP.bypass, replica_groups=[list(range(NC))],
            ins=[ef_in[:]], outs=[ef_full[:]])
        pass_b(ef_full, 'gather', True, attn_t)
        nc.gpsimd.collective_compute(
            "AllGather", OP.bypass, replica_groups=[list(range(NC))],
            ins=[xw2_in[:]], outs=[xw2_full[:]])

        # ---------------- Layer 2 ----------------
        pass_a(hgA2_idx, xw2_full, ef2_in, a2_2)
        nc.gpsimd.collective_compute(
            "AllGather", OP.bypass, replica_groups=[list(range(NC))],
            ins=[ef2_in[:]], outs=[ef2_full[:]])
        pass_b(ef2_full, 'local', False, attn_t)

        nc.sync.dma_start(out=attn_out[:], in_=attn_t[:])

    nc.compile()
    return nc


def kernel(**inputs):
    meta, in_maps, origB = _prep(inputs)
    nc = _build(meta)
    res = bass_utils.run_bass_kernel_spmd(nc, in_maps, core_ids=list(range(NC)))
    h = np.zeros((N_NODE, D), np.float32)
    attn = np.zeros((E2,), np.float32)
    for c in range(NC):
        h[c * NODE_SH:(c + 1) * NODE_SH] = res.results[c]['h_out'][:NODE_SH]
        a = res.results[c]['attn_out']
        m = origB[c]
        valid = m >= 0
        attn[m[valid]] = a[valid]
    return h, attn
